# revision 1
# baseline (speedup 1.0000x reference)
# Tensor-parallel GQA attention kernel for 8 Trainium2 NeuronCores.
#
# Sharding: each core owns 4 query heads + 1 kv head (32 q / 8 kv heads
# total), computes q/k/v projections for its heads, RoPE, causal
# attention, and a partial o_proj (row slice of wo); the host sums the 8
# partial outputs.
#
# Per-core layout: everything is kept "transposed" ([dim, seq]) so the
# contraction dim of every matmul is the partition axis:
#   qT = wq_c.T @ x.T        [256, S]   (scale folded into wq_c)
#   kvT = wkv_c.T @ x.T      [128, S]   (k rows 0:64, v rows 64:128)
#   scoresT[j, i] = kT.T q   [128-block j, 512-chunk i]  (K=64)
#   causal mask: an extra matmul accumulates -80 * (1 - mask) into the
#   scores psum ((-80 I).T @ cminv), so exp() of masked entries ~ 1e-33
#   attn_T[d, i] = [v|1].T @ exp(scores)   (row 64 = softmax denoms)
#   out_partial[s, :] = attn_T.T-contracted with wo_c rows (fp16 out,
#   host accumulates)
# Normalization happens after PV: one batched reciprocal of the 4 heads'
# denominator rows (gathered at 32-aligned partitions), then a K=128
# selector matmul broadcasts them, then an elementwise multiply.
#
# Scheduling notes:
#  - persistent tensors (qr, ktd, vtmp, vb) are split into two seq
#    halves because Tile tracks dependencies per tile: with single
#    tiles the first attention matmul would wait for the last
#    projection write.
#  - one PSUM pool set serves both phases; emission order is
#    proj(half0) -> attention(chunk0) -> proj(half1) -> attention(1..3)
#    so the chunk-0 attention covers projection-half-1's PSUM slot
#    waits and vice versa.
#  - the PV matmuls lag their score matmuls by two J-blocks so the PE
#    never waits on the exp chain, and the previous chunk's o_proj
#    matmuls are spread through the J-loop as dense PE filler (keeps
#    the HAM clock warm through exp-bound stretches).
#  - V is transposed with the DVE 32x32 stream transpose (no PE/PSUM).

import sys
from contextlib import ExitStack

for _p in ("/opt/trn_rl_repo", "/root/.axon_site"):
    if _p not in sys.path:
        sys.path.insert(0, _p)

import numpy as np

import concourse.bacc as bacc
import concourse.mybir as mybir
import concourse.tile as tile
from concourse.bass_utils import run_bass_kernel_spmd

F32 = mybir.dt.float32
EXP = mybir.ActivationFunctionType.Exp

# matmul operand dtype: float32r (2 cyc/row, ~1.5e-4/matmul) or
# float16 (1 cyc/row, ~7e-4/matmul)
MM_DT = mybir.dt.float16
MASK_NEG = 80.0

S = 2048          # sequence length
H = 2048          # hidden size
NH = 32           # query heads
NKV = 8           # kv heads
HD = 64           # head dim
NCORES = 8
HPC = NH // NCORES        # query heads per core = 4
DQ = HPC * HD             # per-core q width = 256
SCALE = HD ** -0.5
P = 128
NB = S // P               # 16 128-blocks along seq
NC4 = S // 512            # 4 512-chunks along seq
KCH = H // P              # 16 contraction chunks
HS = S // 2               # half seq


def _build_module(mm_dt):
    nc = bacc.Bacc(trn_type="TRN2", debug=False)

    xT_d = nc.dram_tensor("xT", [H, S], mm_dt, kind="ExternalInput").ap()
    wq_d = nc.dram_tensor("wq", [P, KCH * DQ], mm_dt, kind="ExternalInput").ap()
    wkv_d = nc.dram_tensor("wkv", [P, KCH * P], mm_dt, kind="ExternalInput").ap()
    wo_d = nc.dram_tensor("wo", [P, 2 * S], mm_dt, kind="ExternalInput").ap()
    cos_d = nc.dram_tensor("cos2", [P, S], F32, kind="ExternalInput").ap()
    sin_d = nc.dram_tensor("sin2", [P, S], F32, kind="ExternalInput").ap()
    # inverted causal masks for the 4 diagonal offsets, and -80*I
    cm_d = nc.dram_tensor("cminv", [P, 4 * 512], mm_dt, kind="ExternalInput").ap()
    ni_d = nc.dram_tensor("negi", [P, P], mm_dt, kind="ExternalInput").ap()
    # selector matrices for the denominator broadcast
    e0_d = nc.dram_tensor("e0", [P, P], mm_dt, kind="ExternalInput").ap()
    e1_d = nc.dram_tensor("e1", [P, P], mm_dt, kind="ExternalInput").ap()
    out_d = nc.dram_tensor("out", [S, H], mm_dt, kind="ExternalOutput").ap()

    with tile.TileContext(nc) as tc, ExitStack() as ctx:
        pers = ctx.enter_context(tc.tile_pool(name="pers", bufs=1))

        wq_sb = pers.tile([P, KCH * DQ], mm_dt, tag="wq_sb", name="wq_sb")
        wkv_sb = pers.tile([P, KCH * P], mm_dt, tag="wkv_sb", name="wkv_sb")
        cos_sb = pers.tile([P, S], F32, tag="cos_sb", name="cos_sb")
        sin_sb = pers.tile([P, S], F32, tag="sin_sb", name="sin_sb")
        wo_sb = pers.tile([P, 2 * S], mm_dt, tag="wo_sb", name="wo_sb")
        cm_sb = pers.tile([P, 4 * 512], mm_dt, tag="cm_sb", name="cm_sb")
        ni_sb = pers.tile([P, P], mm_dt, tag="ni_sb", name="ni_sb")
        e0_sb = pers.tile([P, P], mm_dt, tag="e0_sb", name="e0_sb")
        e1_sb = pers.tile([P, P], mm_dt, tag="e1_sb", name="e1_sb")
        e_sb = [e0_sb, e1_sb]

        ones16 = pers.tile([P, NB], mm_dt, tag="ones16", name="ones16")
        nc.vector.memset(ones16[:], 1.0)

        # per-half persistent tensors
        qrh = [[pers.tile([P, HS], mm_dt, tag=f"qr{m}_{g}", name=f"qr{m}_{g}")
                for g in range(2)] for m in range(2)]
        ktdh = [pers.tile([P, HS], mm_dt, tag=f"ktd{g}", name=f"ktd{g}")
                for g in range(2)]
        vtmph = [pers.tile([64, HS], mm_dt, tag=f"vtmp{g}", name=f"vtmp{g}")
                 for g in range(2)]
        vbh = [pers.tile([P, 8 * (HD + 1)], mm_dt, tag=f"vb{g}", name=f"vb{g}")
               for g in range(2)]
        attn0 = pers.tile([P, S], mm_dt, tag="attn0", name="attn0")
        attn1 = pers.tile([P, S], mm_dt, tag="attn1", name="attn1")
        attn = [attn0, attn1]
        rsum = pers.tile([P, 512], F32, tag="rsum", name="rsum")
        nc.vector.memset(rsum[:], 1.0)
        rrf = pers.tile([P, 512], F32, tag="rrf", name="rrf")
        rrs = pers.tile([P, 512], F32, tag="rrs", name="rrs")
        rr = pers.tile([P, 512], mm_dt, tag="rr", name="rr")

        for g in range(2):
            vbv = vbh[g][:].rearrange("p (b c) -> p b c", c=HD + 1)
            nc.vector.tensor_copy(vbv[:, :, HD:HD + 1], ones16[:, 0:8])

        # shared pools: one PSUM set for both phases
        xp = ctx.enter_context(tc.tile_pool(name="xp", bufs=6))
        rs = ctx.enter_context(tc.tile_pool(name="rs", bufs=2))
        pop = ctx.enter_context(tc.tile_pool(name="pop", bufs=4, space="PSUM"))
        wkp = ctx.enter_context(tc.tile_pool(name="wkp", bufs=4, space="PSUM"))
        pxp = ctx.enter_context(tc.tile_pool(name="pxp", bufs=10))
        otp = ctx.enter_context(tc.tile_pool(name="otp", bufs=4))

        # ---------------- projections + RoPE ----------------
        def proj_half(g):
            nw = slice(1024 * g, 1024 * g + 1024)
            psq = [[None, None], [None, None]]
            pskv = [None, None]
            for half in range(2):
                psq[0][half] = pop.tile([P, 512], F32, tag="pop",
                                        name=f"psq0_{half}")
                psq[1][half] = pop.tile([P, 512], F32, tag="pop",
                                        name=f"psq1_{half}")
                pskv[half] = wkp.tile([P, 512], F32, tag="wk",
                                      name=f"pskv_{half}")
            for k in range(KCH):
                if g == 0 and (k in (0, 1) or (k % 4 == 2 and k < 12)):
                    # k=0/1 come alone so the first matmuls start early
                    if k == 0:
                        wqs, wks = slice(0, 256), slice(0, 128)
                    elif k == 1:
                        wqs, wks = slice(256, 1024), slice(128, 512)
                    else:
                        kg = k // 4 + 1
                        wqs = slice(1024 * kg, 1024 * kg + 1024)
                        wks = slice(512 * kg, 512 * kg + 512)
                    nc.sync.dma_start(wq_sb[:, wqs], wq_d[:, wqs])
                    nc.sync.dma_start(wkv_sb[:, wks], wkv_d[:, wks])
                xt = xp.tile([P, 1024], mm_dt, tag="xt", name="xt")
                nc.sync.dma_start(xt[:], xT_d[P * k:P * k + P, nw])
                st = dict(start=(k == 0), stop=(k == KCH - 1))
                for half in range(2):
                    xs = xt[:, 512 * half:512 * half + 512]
                    nc.tensor.matmul(
                        psq[0][half][:], wq_sb[:, k * DQ:k * DQ + P], xs, **st)
                    nc.tensor.matmul(
                        psq[1][half][:], wq_sb[:, k * DQ + P:k * DQ + DQ],
                        xs, **st)
                    nc.tensor.matmul(
                        pskv[half][:], wkv_sb[:, k * P:k * P + P], xs, **st)
                if g == 0 and k == 3:
                    # trig tables needed by the first RoPE below
                    nc.sync.dma_start(cos_sb[:], cos_d[:])
                    nc.sync.dma_start(sin_sb[:], sin_d[:])
                if g == 0 and k == 9:
                    nc.sync.dma_start(cm_sb[:], cm_d[:])
                    nc.sync.dma_start(ni_sb[:], ni_d[:])
                    nc.sync.dma_start(e0_sb[:], e0_d[:])
                    nc.sync.dma_start(e1_sb[:], e1_d[:])
                    nc.sync.dma_start(wo_sb[:], wo_d[:])

            # RoPE: emit every psum-reading op first so the projection
            # accumulator banks release as early as possible (they gate
            # the attention PSUM slots), then the SBUF-only chain
            qraws = []
            for m in range(2):
                qraw = rs.tile([P, 1024], F32, tag=f"qraw{m}", name=f"qraw{m}")
                qraws.append(qraw)
                for half in range(2):
                    nc.vector.tensor_copy(
                        qraw[:, 512 * half:512 * half + 512], psq[m][half][:])
                for half in range(2):
                    hs = slice(512 * half, 512 * half + 512)
                    nc.vector.tensor_mul(
                        qrh[m][g][:, hs], psq[m][half][:],
                        cos_sb[:, 1024 * g + 512 * half:
                               1024 * g + 512 * half + 512])
            kraw = rs.tile([64, 1024], F32, tag="kraw", name="kraw")
            for half in range(2):
                nc.vector.tensor_copy(
                    kraw[:, 512 * half:512 * half + 512], pskv[half][0:64, :])
            for half in range(2):
                hs = slice(512 * half, 512 * half + 512)
                nc.vector.tensor_mul(
                    ktdh[g][0:64, hs], pskv[half][0:64, :],
                    cos_sb[0:64, 1024 * g + 512 * half:
                           1024 * g + 512 * half + 512])
            for half in range(2):
                hs = slice(512 * half, 512 * half + 512)
                nc.vector.tensor_copy(vtmph[g][:, hs], pskv[half][64:P, :])
            # SBUF-only rotate-half chains
            for m in range(2):
                qraw = qraws[m]
                qsw = rs.tile([P, 1024], F32, tag=f"qsw{m}", name=f"qsw{m}")
                for b0 in (0, 64):
                    nc.vector.tensor_copy(
                        qsw[b0:b0 + 32, :], qraw[b0 + 32:b0 + 64, :])
                    nc.vector.tensor_copy(
                        qsw[b0 + 32:b0 + 64, :], qraw[b0:b0 + 32, :])
                nc.vector.tensor_mul(qsw[:], qsw[:], sin_sb[:, nw])
                nc.vector.tensor_add(qrh[m][g][:], qrh[m][g][:], qsw[:])
            ksw = rs.tile([64, 1024], F32, tag="ksw", name="ksw")
            nc.vector.tensor_copy(ksw[0:32, :], kraw[32:64, :])
            nc.vector.tensor_copy(ksw[32:64, :], kraw[0:32, :])
            nc.vector.tensor_mul(ksw[:], ksw[:], sin_sb[0:64, nw])
            nc.vector.tensor_add(ktdh[g][0:64, :], ktdh[g][0:64, :], ksw[:])
            # duplicate k rows for the upper-head score matmuls
            nc.vector.tensor_copy(ktdh[g][64:P, :], ktdh[g][0:64, :])
            # stream-transpose v into vb blocks (DVE)
            vbv = vbh[g][:].rearrange("p (b c) -> p b c", c=HD + 1)
            vtv = vtmph[g][:].rearrange("p (b c) -> p b c", c=P)
            for a in range(4):
                for b in range(2):
                    nc.vector.transpose(
                        vbv[32 * a:32 * a + 32, :, 32 * b:32 * b + 32],
                        vtv[32 * b:32 * b + 32, :, 32 * a:32 * a + 32])

        # ---------------- attention + o_proj ----------------
        # software-pipelined: PV matmuls lag their score matmuls by two
        # J-blocks so the PE never waits on the exp chain
        pxstore = {}

        def emit_scores(ic, J, po):
            icg, icr = ic // 2, ic % 2
            t = J - 4 * ic
            c0 = 128 * t if t > 0 else 0
            Jg, Jr = J // 8, J % 8
            Js = slice(P * Jr, P * Jr + P)
            pxs = []
            for h in range(HPC):
                m, b0 = h // 2, 64 * (h % 2)
                qs = slice(512 * icr + c0, 512 * icr + 512)
                ps_s = wkp.tile([P, 512], F32, tag="wk", name="ps_s")
                nc.tensor.matmul(
                    ps_s[:, c0:], ktdh[Jg][b0:b0 + 64, Js],
                    qrh[m][icg][b0:b0 + 64, qs],
                    start=True, stop=(t < 0))
                if t >= 0:
                    # accumulate -80 * inverted causal mask
                    nc.tensor.matmul(
                        ps_s[:, c0:], ni_sb[:],
                        cm_sb[:, 512 * t + c0:512 * t + 512],
                        start=False, stop=True)
                px = pxp.tile([P, 512], mm_dt, tag="pxp", name="px")
                nc.scalar.activation(px[:, c0:], ps_s[:, c0:], EXP)
                pxs.append(px)
            pxstore[(ic, J)] = pxs

        def emit_pv(ic, J, po):
            t = J - 4 * ic
            c0 = 128 * t if t > 0 else 0
            nJ = 4 * ic + 4
            Jg, Jr = J // 8, J % 8
            vs = slice((HD + 1) * Jr, (HD + 1) * Jr + HD + 1)
            pxs = pxstore.pop((ic, J))
            for h in range(HPC):
                nc.tensor.matmul(
                    po[h][:, c0:], vbh[Jg][:, vs], pxs[h][:, c0:],
                    start=(J == 0), stop=(J == nJ - 1))

        def emit_oproj_unit(sb, n4):
            ss = slice(P * sb, P * sb + P)
            ps_o = wkp.tile([P, 512], F32, tag="wk", name="ps_o")
            nc.tensor.matmul(
                ps_o[:], attn0[:, ss],
                wo_sb[:, 512 * n4:512 * n4 + 512],
                start=True, stop=False)
            nc.tensor.matmul(
                ps_o[:], attn1[:, ss],
                wo_sb[:, S + 512 * n4:S + 512 * n4 + 512],
                start=False, stop=True)
            ot = otp.tile([P, 512], mm_dt, tag="otp", name="ot")
            nc.vector.tensor_copy(ot[:], ps_o[:])
            nc.sync.dma_start(out_d[ss, 512 * n4:512 * n4 + 512], ot[:])

        def finalize_ic(ic, po):
            # batched softmax denominators: gather the 4 heads' rows at
            # 32-aligned partitions, one fast reciprocal, then a K=128
            # selector-matmul broadcast and normalize
            for h in range(HPC):
                nc.vector.tensor_copy(
                    rsum[32 * h:32 * h + 1, :], po[h][HD:HD + 1, :])
            with nc.allow_low_precision(reason="softmax reciprocal"):
                nc.vector.reciprocal_approx_accurate(rrf[:], rsum[:], rrs[:])
                nc.vector.tensor_copy(rr[:], rrf[:])
            for m in range(2):
                psb = wkp.tile([P, 512], F32, tag="wk", name="psb")
                nc.tensor.matmul(psb[:], e_sb[m][:], rr[:],
                                 start=True, stop=True)
                for hh in range(2):
                    h, b0 = 2 * m + hh, 64 * hh
                    asl = attn[m][b0:b0 + 64, 512 * ic:512 * ic + 512]
                    nc.vector.tensor_copy(asl, po[h][0:HD, :])
                    nc.vector.tensor_mul(asl, asl, psb[b0:b0 + 64, :])

        def attn_ic(ic, po, po_next, oproj_prev):
            # interleave the previous chunk's o_proj matmuls into this
            # J-loop as dense PE filler
            nJ = 4 * ic + 4
            allu = list(oproj_prev)
            units, tailu = (allu[:-3], allu[-3:]) if allu else ([], [])
            ui = 0
            for J in range(2, nJ):
                emit_scores(ic, J, po)
                emit_pv(ic, J - 2, po)
                take = ((len(units) * (J - 1)) // (nJ - 2)) - ui
                while take > 0 and ui < len(units):
                    emit_oproj_unit(*units[ui]); ui += 1; take -= 1
            while ui < len(units):
                emit_oproj_unit(*units[ui]); ui += 1
            emit_pv(ic, nJ - 2, po)
            emit_pv(ic, nJ - 1, po)
            # cover the reciprocal latency with the next chunk's scores
            # plus a few held-back o_proj units
            if po_next is not None:
                emit_scores(ic + 1, 0, po_next)
                emit_scores(ic + 1, 1, po_next)
            for u in tailu:
                emit_oproj_unit(*u)
            finalize_ic(ic, po)

        def oproj_units(ic):
            return [(sb, n4) for sb in range(4 * ic, 4 * ic + 4)
                    for n4 in range(NC4)]

        proj_half(0)
        proj_half(1)
        pos = [
            [pop.tile([HD + 1, 512], F32, tag="pop", name=f"po{ic}_{h}")
             for h in range(HPC)]
            for ic in range(NC4)
        ]
        emit_scores(0, 0, pos[0])
        emit_scores(0, 1, pos[0])
        for ic in range(NC4):
            po_next = pos[ic + 1] if ic + 1 < NC4 else None
            attn_ic(ic, pos[ic], po_next,
                    oproj_units(ic - 1) if ic > 0 else [])
        for sb in range(12, 16):
            for n4 in range(NC4):
                emit_oproj_unit(sb, n4)

    nc.compile()
    return nc


_NC_CACHE = {}


def _get_module(mm_dt=MM_DT):
    if mm_dt not in _NC_CACHE:
        _NC_CACHE[mm_dt] = _build_module(mm_dt)
    return _NC_CACHE[mm_dt]


def _prep_inputs(x, wq, wk, wv, wo, cos, sin, mm_dt=MM_DT):
    mm_np = mybir.dt.np(mm_dt)
    x = np.asarray(x, dtype=np.float32)
    xT = np.ascontiguousarray(x.reshape(S, H).T.astype(mm_np))

    cosT = np.asarray(cos, dtype=np.float32).T          # [64, S]
    sinT = np.asarray(sin, dtype=np.float32).T          # [64, S]
    sgn = np.where(np.arange(HD) < HD // 2, -1.0, 1.0).astype(np.float32)
    sinT_s = sinT * sgn[:, None]
    cos2 = np.ascontiguousarray(np.tile(cosT, (2, 1)))  # [128, S]
    sin2 = np.ascontiguousarray(np.tile(sinT_s, (2, 1)))

    # inverted causal masks (1 where masked out), diagonal offsets 0..3
    jl = np.arange(P)[:, None]
    il = np.arange(512)[None, :]
    cminv = np.concatenate(
        [(jl + P * t > il).astype(np.float32) for t in range(4)], axis=1)
    cminv = np.ascontiguousarray(cminv).astype(mm_np)
    negi = (-MASK_NEG * np.eye(P, dtype=np.float32)).astype(mm_np)

    # selector matrices: psb_m rows 0:64 get the reciprocal row of head
    # 2m (partition 64m), rows 64:128 head 2m+1 (partition 64m+32)
    e0 = np.zeros((P, P), dtype=np.float32)
    e1 = np.zeros((P, P), dtype=np.float32)
    e0[0, 0:64] = 1.0
    e0[32, 64:128] = 1.0
    e1[64, 0:64] = 1.0
    e1[96, 64:128] = 1.0
    e0 = e0.astype(mm_np)
    e1 = e1.astype(mm_np)

    def chunk_kxm(w):
        # [H, M] -> [128, KCH*M] with k-chunk-major free layout
        m = w.shape[1]
        return np.ascontiguousarray(
            w.reshape(KCH, P, m).transpose(1, 0, 2).reshape(P, KCH * m).astype(mm_np))

    wq = np.asarray(wq, dtype=np.float32)
    wk = np.asarray(wk, dtype=np.float32)
    wv = np.asarray(wv, dtype=np.float32)
    wo = np.asarray(wo, dtype=np.float32)

    in_maps = []
    for c in range(NCORES):
        wq_c = wq[:, DQ * c:DQ * c + DQ] * SCALE
        wkv_c = np.concatenate(
            [wk[:, HD * c:HD * c + HD], wv[:, HD * c:HD * c + HD]], axis=1)
        wo_c = wo[DQ * c:DQ * c + DQ, :]
        wo_l = np.ascontiguousarray(
            wo_c.reshape(2, P, H).transpose(1, 0, 2).reshape(P, 2 * H).astype(mm_np))
        in_maps.append({
            "xT": xT,
            "wq": chunk_kxm(wq_c),
            "wkv": chunk_kxm(wkv_c),
            "wo": wo_l,
            "cos2": cos2,
            "sin2": sin2,
            "cminv": cminv,
            "negi": negi,
            "e0": e0,
            "e1": e1,
        })
    return in_maps


def run(inputs, trace=False, trace_kwargs=None, mm_dt=MM_DT):
    """Execute on 8 cores; returns (full_output, BassKernelResults)."""
    nc = _get_module(mm_dt)
    in_maps = _prep_inputs(
        inputs["x"], inputs["wq"], inputs["wk"], inputs["wv"],
        inputs["wo"], inputs["cos"], inputs["sin"], mm_dt=mm_dt)
    kwargs = {}
    if trace:
        kwargs = dict(trace=True, **(trace_kwargs or {}))
    res = run_bass_kernel_spmd(nc, in_maps, core_ids=list(range(NCORES)), **kwargs)
    acc = np.zeros((S, H), dtype=np.float32)
    for c in range(NCORES):
        acc += res.results[c]["out"].astype(np.float32)
    out = acc.reshape(1, S, H)
    return out, res


def kernel(**inputs):
    out, _ = run(inputs, trace=False)
    return out



# revision 9
# speedup vs baseline: 1.0852x; 1.0852x over previous
# Tensor-parallel GQA attention kernel for 8 Trainium2 NeuronCores.
#
# Sharding: each core owns 4 query heads + 1 kv head (32 q / 8 kv heads
# total), computes q/k/v projections for its heads, RoPE, causal
# attention, and a partial o_proj (row slice of wo); the host sums the 8
# partial outputs.
#
# Per-core layout: everything is kept "transposed" ([dim, seq]) so the
# contraction dim of every matmul is the partition axis:
#   qT = wq_c.T @ x.T        [256, S]   (scale folded into wq_c)
#   kvT = wkv_c.T @ x.T      [128, S]   (k rows 0:64, v rows 64:128)
#   scoresT[j, i] = kT.T q   [128-block j, 512-chunk i]  (K=64)
#   causal mask: an extra matmul accumulates -80 * (1 - mask) into the
#   scores psum ((-80 I).T @ cminv), so exp() of masked entries ~ 1e-33
#   attn_T[d, i] = [v|1].T @ exp(scores)   (row 64 = softmax denoms)
#   out_partial[s, :] = attn_T.T-contracted with wo_c rows (fp16 out,
#   host accumulates)
# Normalization happens after PV: one batched reciprocal of the 4 heads'
# denominator rows (gathered at 32-aligned partitions), then a K=128
# selector matmul broadcasts them, then an elementwise multiply.
#
# Scheduling notes:
#  - persistent tensors (qr, ktd, vtmp, vb) are split into two seq
#    halves because Tile tracks dependencies per tile: with single
#    tiles the first attention matmul would wait for the last
#    projection write.
#  - one PSUM pool set serves both phases; emission order is
#    proj(half0) -> attention(chunk0) -> proj(half1) -> attention(1..3)
#    so the chunk-0 attention covers projection-half-1's PSUM slot
#    waits and vice versa.
#  - the PV matmuls lag their score matmuls by two J-blocks so the PE
#    never waits on the exp chain, and the previous chunk's o_proj
#    matmuls are spread through the J-loop as dense PE filler (keeps
#    the HAM clock warm through exp-bound stretches).
#  - V is transposed with the DVE 32x32 stream transpose (no PE/PSUM).

import sys
from contextlib import ExitStack

for _p in ("/opt/trn_rl_repo", "/root/.axon_site"):
    if _p not in sys.path:
        sys.path.insert(0, _p)

import numpy as np

import concourse.bacc as bacc
import concourse.mybir as mybir
import concourse.tile as tile
from concourse.bass_utils import run_bass_kernel_spmd

F32 = mybir.dt.float32
EXP = mybir.ActivationFunctionType.Exp

# matmul operand dtype: float32r (2 cyc/row, ~1.5e-4/matmul) or
# float16 (1 cyc/row, ~7e-4/matmul)
MM_DT = mybir.dt.float16
MASK_NEG = 80.0

S = 2048          # sequence length
H = 2048          # hidden size
NH = 32           # query heads
NKV = 8           # kv heads
HD = 64           # head dim
NCORES = 8
HPC = NH // NCORES        # query heads per core = 4
DQ = HPC * HD             # per-core q width = 256
SCALE = HD ** -0.5
P = 128
NB = S // P               # 16 128-blocks along seq
NC4 = S // 512            # 4 512-chunks along seq
KCH = H // P              # 16 contraction chunks
HS = S // 2               # half seq


def _build_module(mm_dt):
    nc = bacc.Bacc(trn_type="TRN2", debug=False)

    xT_d = nc.dram_tensor("xT", [H, S], mm_dt, kind="ExternalInput").ap()
    wq_d = nc.dram_tensor("wq", [P, KCH * DQ], mm_dt, kind="ExternalInput").ap()
    wkv_d = nc.dram_tensor("wkv", [P, KCH * P], mm_dt, kind="ExternalInput").ap()
    wo_d = nc.dram_tensor("wo", [P, 2 * S], mm_dt, kind="ExternalInput").ap()
    cos_d = nc.dram_tensor("cos2", [P, S], mm_dt, kind="ExternalInput").ap()
    sin_d = nc.dram_tensor("sin2", [P, S], mm_dt, kind="ExternalInput").ap()
    # inverted causal masks for the 4 diagonal offsets, and -80*I
    cm_d = nc.dram_tensor("cminv", [P, 4 * 512], mm_dt, kind="ExternalInput").ap()
    ni_d = nc.dram_tensor("negi", [P, P], mm_dt, kind="ExternalInput").ap()
    # selector matrices for the denominator broadcast
    e0_d = nc.dram_tensor("e0", [P, P], mm_dt, kind="ExternalInput").ap()
    e1_d = nc.dram_tensor("e1", [P, P], mm_dt, kind="ExternalInput").ap()
    out_d = nc.dram_tensor("out", [S, H], mm_dt, kind="ExternalOutput").ap()

    with tile.TileContext(nc) as tc, ExitStack() as ctx:
        pers = ctx.enter_context(tc.tile_pool(name="pers", bufs=1))

        wq_sb = pers.tile([P, KCH * DQ], mm_dt, tag="wq_sb", name="wq_sb")
        wkv_sb = pers.tile([P, KCH * P], mm_dt, tag="wkv_sb", name="wkv_sb")
        cos_sb = pers.tile([P, S], mm_dt, tag="cos_sb", name="cos_sb")
        sin_sb = pers.tile([P, S], mm_dt, tag="sin_sb", name="sin_sb")
        wo_sb = pers.tile([P, 2 * S], mm_dt, tag="wo_sb", name="wo_sb")
        cm_sb = pers.tile([P, 4 * 512], mm_dt, tag="cm_sb", name="cm_sb")
        ni_sb = pers.tile([P, P], mm_dt, tag="ni_sb", name="ni_sb")
        e0_sb = pers.tile([P, P], mm_dt, tag="e0_sb", name="e0_sb")
        e1_sb = pers.tile([P, P], mm_dt, tag="e1_sb", name="e1_sb")
        e_sb = [e0_sb, e1_sb]

        ones16 = pers.tile([P, NB], mm_dt, tag="ones16", name="ones16")
        nc.vector.memset(ones16[:], 1.0)

        # per-half persistent tensors
        qrh = [[pers.tile([P, HS], mm_dt, tag=f"qr{m}_{g}", name=f"qr{m}_{g}")
                for g in range(2)] for m in range(2)]
        ktdh = [pers.tile([P, HS], mm_dt, tag=f"ktd{g}", name=f"ktd{g}")
                for g in range(2)]
        vtmph = [pers.tile([64, HS], mm_dt, tag=f"vtmp{g}", name=f"vtmp{g}")
                 for g in range(2)]
        vbh = [pers.tile([P, 8 * (HD + 1)], mm_dt, tag=f"vb{g}", name=f"vb{g}")
               for g in range(2)]
        attn0 = pers.tile([P, S], mm_dt, tag="attn0", name="attn0")
        attn1 = pers.tile([P, S], mm_dt, tag="attn1", name="attn1")
        attn = [attn0, attn1]
        rsum = pers.tile([P, 512], F32, tag="rsum", name="rsum")
        nc.vector.memset(rsum[:], 1.0)
        rrf = pers.tile([P, 512], F32, tag="rrf", name="rrf")
        rrs = pers.tile([P, 512], F32, tag="rrs", name="rrs")
        rr = pers.tile([P, 512], mm_dt, tag="rr", name="rr")

        for g in range(2):
            vbv = vbh[g][:].rearrange("p (b c) -> p b c", c=HD + 1)
            nc.vector.tensor_copy(vbv[:, :, HD:HD + 1], ones16[:, 0:8])

        # shared pools: one PSUM set for both phases
        xp = ctx.enter_context(tc.tile_pool(name="xp", bufs=6))
        rs = ctx.enter_context(tc.tile_pool(name="rs", bufs=2))
        pop = ctx.enter_context(tc.tile_pool(name="pop", bufs=4, space="PSUM"))
        wkp = ctx.enter_context(tc.tile_pool(name="wkp", bufs=4, space="PSUM"))
        pxp = ctx.enter_context(tc.tile_pool(name="pxp", bufs=10))
        otp = ctx.enter_context(tc.tile_pool(name="otp", bufs=4))

        # ---------------- projections + RoPE ----------------
        def proj_half(g):
            nw = slice(1024 * g, 1024 * g + 1024)
            psq = [[None, None], [None, None]]
            pskv = [None, None]
            for half in range(2):
                psq[0][half] = pop.tile([P, 512], F32, tag="pop",
                                        name=f"psq0_{half}")
                psq[1][half] = pop.tile([P, 512], F32, tag="pop",
                                        name=f"psq1_{half}")
                pskv[half] = wkp.tile([P, 512], F32, tag="wk",
                                      name=f"pskv_{half}")
            for k in range(KCH):
                if g == 0 and (k in (0, 1) or (k % 4 == 2 and k < 12)):
                    # k=0/1 come alone so the first matmuls start early
                    if k == 0:
                        wqs, wks = slice(0, 256), slice(0, 128)
                    elif k == 1:
                        wqs, wks = slice(256, 1024), slice(128, 512)
                    else:
                        kg = k // 4 + 1
                        wqs = slice(1024 * kg, 1024 * kg + 1024)
                        wks = slice(512 * kg, 512 * kg + 512)
                    nc.sync.dma_start(wq_sb[:, wqs], wq_d[:, wqs])
                    nc.sync.dma_start(wkv_sb[:, wks], wkv_d[:, wks])
                xt = xp.tile([P, 1024], mm_dt, tag="xt", name="xt")
                if g == 0 and k < 2:
                    # split the first x tiles so the first matmul's data
                    # lands sooner
                    nc.sync.dma_start(xt[:, 0:512],
                                      xT_d[P * k:P * k + P, 1024 * g:1024 * g + 512])
                    nc.sync.dma_start(xt[:, 512:1024],
                                      xT_d[P * k:P * k + P, 1024 * g + 512:1024 * g + 1024])
                else:
                    nc.sync.dma_start(xt[:], xT_d[P * k:P * k + P, nw])
                st = dict(start=(k == 0), stop=(k == KCH - 1))
                for half in range(2):
                    xs = xt[:, 512 * half:512 * half + 512]
                    nc.tensor.matmul(
                        psq[0][half][:], wq_sb[:, k * DQ:k * DQ + P], xs, **st)
                    nc.tensor.matmul(
                        psq[1][half][:], wq_sb[:, k * DQ + P:k * DQ + DQ],
                        xs, **st)
                    nc.tensor.matmul(
                        pskv[half][:], wkv_sb[:, k * P:k * P + P], xs, **st)
                if g == 0 and k == 3:
                    # trig tables needed by the first RoPE below
                    nc.sync.dma_start(cos_sb[:], cos_d[:])
                    nc.sync.dma_start(sin_sb[:], sin_d[:])
                if g == 0 and k == 9:
                    nc.sync.dma_start(cm_sb[:], cm_d[:])
                    nc.sync.dma_start(ni_sb[:], ni_d[:])
                    nc.sync.dma_start(e0_sb[:], e0_d[:])
                    nc.sync.dma_start(e1_sb[:], e1_d[:])
                    nc.sync.dma_start(wo_sb[:], wo_d[:])

            # RoPE: emit every psum-reading op first so the projection
            # accumulator banks release as early as possible (they gate
            # the attention PSUM slots).  PSUM evacuation is split:
            # DVE does the cos-multiplies, the Scalar engine (idle during
            # projections) does the raw fp16 casts, then an fp16
            # SBUF-only rotate-half chain on DVE.
            q16s = []
            for m in range(2):
                q16 = rs.tile([P, 1024], mm_dt, tag=f"q16_{m}", name=f"q16_{m}")
                q16s.append(q16)
                for half in range(2):
                    hs = slice(512 * half, 512 * half + 512)
                    nc.vector.tensor_mul(
                        qrh[m][g][:, hs], psq[m][half][:],
                        cos_sb[:, 1024 * g + 512 * half:
                               1024 * g + 512 * half + 512])
                for half in range(2):
                    hs = slice(512 * half, 512 * half + 512)
                    nc.scalar.copy(q16[:, hs], psq[m][half][:])
            k16 = rs.tile([64, 1024], mm_dt, tag="k16", name="k16")
            for half in range(2):
                hs = slice(512 * half, 512 * half + 512)
                nc.vector.tensor_mul(
                    ktdh[g][0:64, hs], pskv[half][0:64, :],
                    cos_sb[0:64, 1024 * g + 512 * half:
                           1024 * g + 512 * half + 512])
            for half in range(2):
                hs = slice(512 * half, 512 * half + 512)
                nc.scalar.copy(k16[:, hs], pskv[half][0:64, :])
            for half in range(2):
                hs = slice(512 * half, 512 * half + 512)
                nc.scalar.copy(vtmph[g][:, hs], pskv[half][64:P, :])
            # SBUF-only fp16 rotate-half chains
            for m in range(2):
                q16 = q16s[m]
                qsw = rs.tile([P, 1024], mm_dt, tag=f"qsw{m}", name=f"qsw{m}")
                for b0 in (0, 64):
                    nc.vector.tensor_copy(
                        qsw[b0:b0 + 32, :], q16[b0 + 32:b0 + 64, :])
                    nc.vector.tensor_copy(
                        qsw[b0 + 32:b0 + 64, :], q16[b0:b0 + 32, :])
                nc.vector.tensor_mul(qsw[:], qsw[:], sin_sb[:, nw])
                nc.vector.tensor_add(qrh[m][g][:], qrh[m][g][:], qsw[:])
            ksw = rs.tile([64, 1024], mm_dt, tag="ksw", name="ksw")
            nc.vector.tensor_copy(ksw[0:32, :], k16[32:64, :])
            nc.vector.tensor_copy(ksw[32:64, :], k16[0:32, :])
            nc.vector.tensor_mul(ksw[:], ksw[:], sin_sb[0:64, nw])
            nc.vector.tensor_add(ktdh[g][0:64, :], ktdh[g][0:64, :], ksw[:])
            # duplicate k rows for the upper-head score matmuls
            nc.vector.tensor_copy(ktdh[g][64:P, :], ktdh[g][0:64, :])
            # stream-transpose v into vb blocks (DVE)
            vbv = vbh[g][:].rearrange("p (b c) -> p b c", c=HD + 1)
            vtv = vtmph[g][:].rearrange("p (b c) -> p b c", c=P)
            for a in range(4):
                for b in range(2):
                    nc.vector.transpose(
                        vbv[32 * a:32 * a + 32, :, 32 * b:32 * b + 32],
                        vtv[32 * b:32 * b + 32, :, 32 * a:32 * a + 32])

        # ---------------- attention + o_proj ----------------
        # software-pipelined: PV matmuls lag their score matmuls by two
        # J-blocks so the PE never waits on the exp chain
        pxstore = {}

        def emit_scores(ic, J, po):
            icg, icr = ic // 2, ic % 2
            t = J - 4 * ic
            c0 = 128 * t if t > 0 else 0
            Jg, Jr = J // 8, J % 8
            Js = slice(P * Jr, P * Jr + P)
            pxs = []
            for h in range(HPC):
                m, b0 = h // 2, 64 * (h % 2)
                qs = slice(512 * icr + c0, 512 * icr + 512)
                ps_s = wkp.tile([P, 512], F32, tag="wk", name="ps_s")
                nc.tensor.matmul(
                    ps_s[:, c0:], ktdh[Jg][b0:b0 + 64, Js],
                    qrh[m][icg][b0:b0 + 64, qs],
                    start=True, stop=(t < 0))
                if t >= 0:
                    # accumulate -80 * inverted causal mask
                    nc.tensor.matmul(
                        ps_s[:, c0:], ni_sb[:],
                        cm_sb[:, 512 * t + c0:512 * t + 512],
                        start=False, stop=True)
                px = pxp.tile([P, 512], mm_dt, tag="pxp", name="px")
                nc.scalar.activation(px[:, c0:], ps_s[:, c0:], EXP)
                pxs.append(px)
            pxstore[(ic, J)] = pxs

        def emit_pv(ic, J, po):
            t = J - 4 * ic
            c0 = 128 * t if t > 0 else 0
            nJ = 4 * ic + 4
            Jg, Jr = J // 8, J % 8
            vs = slice((HD + 1) * Jr, (HD + 1) * Jr + HD + 1)
            pxs = pxstore.pop((ic, J))
            for h in range(HPC):
                nc.tensor.matmul(
                    po[h][:, c0:], vbh[Jg][:, vs], pxs[h][:, c0:],
                    start=(J == 0), stop=(J == nJ - 1))

        ot_store = {}

        def emit_oproj_unit(sb, n4):
            ss = slice(P * sb, P * sb + P)
            ps_o = wkp.tile([P, 512], F32, tag="wk", name="ps_o")
            nc.tensor.matmul(
                ps_o[:], attn0[:, ss],
                wo_sb[:, 512 * n4:512 * n4 + 512],
                start=True, stop=False)
            nc.tensor.matmul(
                ps_o[:], attn1[:, ss],
                wo_sb[:, S + 512 * n4:S + 512 * n4 + 512],
                start=False, stop=True)
            if n4 == 0:
                ot_store[sb] = otp.tile([P, H], mm_dt, tag="otp", name="ot")
            ot = ot_store[sb]
            nc.vector.tensor_copy(ot[:, 512 * n4:512 * n4 + 512], ps_o[:])
            if n4 == NC4 - 1:
                # one batched DMA per 128-row output block
                nc.sync.dma_start(out_d[ss, :], ot_store.pop(sb)[:])

        def finalize_ic(ic, po):
            # batched softmax denominators: gather the 4 heads' rows at
            # 32-aligned partitions, one fast reciprocal, then a K=128
            # selector-matmul broadcast and normalize
            for h in range(HPC):
                nc.vector.tensor_copy(
                    rsum[32 * h:32 * h + 1, :], po[h][HD:HD + 1, :])
            with nc.allow_low_precision(reason="softmax reciprocal"):
                nc.vector.reciprocal_approx_accurate(rrf[:], rsum[:], rrs[:])
                nc.vector.tensor_copy(rr[:], rrf[:])
            for m in range(2):
                psb = wkp.tile([P, 512], F32, tag="wk", name="psb")
                nc.tensor.matmul(psb[:], e_sb[m][:], rr[:],
                                 start=True, stop=True)
                # DVE reads at most one PSUM operand: stage psb in SBUF
                psbs = rs.tile([P, 512], F32, tag="psbs", name="psbs")
                nc.vector.tensor_copy(psbs[:], psb[:])
                for hh in range(2):
                    h, b0 = 2 * m + hh, 64 * hh
                    asl = attn[m][b0:b0 + 64, 512 * ic:512 * ic + 512]
                    nc.vector.tensor_mul(asl, po[h][0:HD, :], psbs[b0:b0 + 64, :])

        def attn_ic(ic, po, po_next, oproj_prev):
            # interleave the previous chunk's o_proj matmuls into this
            # J-loop as dense PE filler
            nJ = 4 * ic + 4
            allu = list(oproj_prev)
            units, tailu = (allu[:-3], allu[-3:]) if allu else ([], [])
            ui = 0
            for J in range(2, nJ):
                emit_scores(ic, J, po)
                emit_pv(ic, J - 2, po)
                take = ((len(units) * (J - 1)) // (nJ - 2)) - ui
                while take > 0 and ui < len(units):
                    emit_oproj_unit(*units[ui]); ui += 1; take -= 1
            while ui < len(units):
                emit_oproj_unit(*units[ui]); ui += 1
            emit_pv(ic, nJ - 2, po)
            emit_pv(ic, nJ - 1, po)
            # cover the reciprocal latency with the next chunk's scores
            # plus a few held-back o_proj units
            if po_next is not None:
                emit_scores(ic + 1, 0, po_next)
                emit_scores(ic + 1, 1, po_next)
            for u in tailu:
                emit_oproj_unit(*u)
            finalize_ic(ic, po)

        def oproj_units(ic):
            return [(sb, n4) for sb in range(4 * ic, 4 * ic + 4)
                    for n4 in range(NC4)]

        proj_half(0)
        proj_half(1)
        pos = [
            [pop.tile([HD + 1, 512], F32, tag="pop", name=f"po{ic}_{h}")
             for h in range(HPC)]
            for ic in range(NC4)
        ]
        emit_scores(0, 0, pos[0])
        emit_scores(0, 1, pos[0])
        for ic in range(NC4):
            po_next = pos[ic + 1] if ic + 1 < NC4 else None
            attn_ic(ic, pos[ic], po_next,
                    oproj_units(ic - 1) if ic > 0 else [])
        for sb in range(12, 16):
            for n4 in range(NC4):
                emit_oproj_unit(sb, n4)

    nc.compile()
    return nc


_NC_CACHE = {}


def _get_module(mm_dt=MM_DT):
    if mm_dt not in _NC_CACHE:
        _NC_CACHE[mm_dt] = _build_module(mm_dt)
    return _NC_CACHE[mm_dt]


def _prep_inputs(x, wq, wk, wv, wo, cos, sin, mm_dt=MM_DT):
    mm_np = mybir.dt.np(mm_dt)
    x = np.asarray(x, dtype=np.float32)
    xT = np.ascontiguousarray(x.reshape(S, H).T.astype(mm_np))

    cosT = np.asarray(cos, dtype=np.float32).T          # [64, S]
    sinT = np.asarray(sin, dtype=np.float32).T          # [64, S]
    sgn = np.where(np.arange(HD) < HD // 2, -1.0, 1.0).astype(np.float32)
    sinT_s = sinT * sgn[:, None]
    cos2 = np.ascontiguousarray(np.tile(cosT, (2, 1))).astype(mm_np)  # [128, S]
    sin2 = np.ascontiguousarray(np.tile(sinT_s, (2, 1))).astype(mm_np)

    # inverted causal masks (1 where masked out), diagonal offsets 0..3
    jl = np.arange(P)[:, None]
    il = np.arange(512)[None, :]
    cminv = np.concatenate(
        [(jl + P * t > il).astype(np.float32) for t in range(4)], axis=1)
    cminv = np.ascontiguousarray(cminv).astype(mm_np)
    negi = (-MASK_NEG * np.eye(P, dtype=np.float32)).astype(mm_np)

    # selector matrices: psb_m rows 0:64 get the reciprocal row of head
    # 2m (partition 64m), rows 64:128 head 2m+1 (partition 64m+32)
    e0 = np.zeros((P, P), dtype=np.float32)
    e1 = np.zeros((P, P), dtype=np.float32)
    e0[0, 0:64] = 1.0
    e0[32, 64:128] = 1.0
    e1[64, 0:64] = 1.0
    e1[96, 64:128] = 1.0
    e0 = e0.astype(mm_np)
    e1 = e1.astype(mm_np)

    def chunk_kxm(w):
        # [H, M] -> [128, KCH*M] with k-chunk-major free layout
        m = w.shape[1]
        return np.ascontiguousarray(
            w.reshape(KCH, P, m).transpose(1, 0, 2).reshape(P, KCH * m).astype(mm_np))

    wq = np.asarray(wq, dtype=np.float32)
    wk = np.asarray(wk, dtype=np.float32)
    wv = np.asarray(wv, dtype=np.float32)
    wo = np.asarray(wo, dtype=np.float32)

    in_maps = []
    for c in range(NCORES):
        wq_c = wq[:, DQ * c:DQ * c + DQ] * SCALE
        wkv_c = np.concatenate(
            [wk[:, HD * c:HD * c + HD], wv[:, HD * c:HD * c + HD]], axis=1)
        wo_c = wo[DQ * c:DQ * c + DQ, :]
        wo_l = np.ascontiguousarray(
            wo_c.reshape(2, P, H).transpose(1, 0, 2).reshape(P, 2 * H).astype(mm_np))
        in_maps.append({
            "xT": xT,
            "wq": chunk_kxm(wq_c),
            "wkv": chunk_kxm(wkv_c),
            "wo": wo_l,
            "cos2": cos2,
            "sin2": sin2,
            "cminv": cminv,
            "negi": negi,
            "e0": e0,
            "e1": e1,
        })
    return in_maps


def run(inputs, trace=False, trace_kwargs=None, mm_dt=MM_DT):
    """Execute on 8 cores; returns (full_output, BassKernelResults)."""
    nc = _get_module(mm_dt)
    in_maps = _prep_inputs(
        inputs["x"], inputs["wq"], inputs["wk"], inputs["wv"],
        inputs["wo"], inputs["cos"], inputs["sin"], mm_dt=mm_dt)
    kwargs = {}
    if trace:
        kwargs = dict(trace=True, **(trace_kwargs or {}))
    res = run_bass_kernel_spmd(nc, in_maps, core_ids=list(range(NCORES)), **kwargs)
    acc = np.zeros((S, H), dtype=np.float32)
    for c in range(NCORES):
        acc += res.results[c]["out"].astype(np.float32)
    out = acc.reshape(1, S, H)
    return out, res


def kernel(**inputs):
    out, _ = run(inputs, trace=False)
    return out



# revision 16
# speedup vs baseline: 1.1809x; 1.0882x over previous
# Tensor-parallel GQA attention kernel for 8 Trainium2 NeuronCores.
#
# Sharding: each core owns 4 query heads + 1 kv head (32 q / 8 kv heads
# total), computes q/k/v projections for its heads, RoPE, causal
# attention, and a partial o_proj (row slice of wo); the host sums the 8
# partial outputs.
#
# Per-core layout: everything is kept "transposed" ([dim, seq]) so the
# contraction dim of every matmul is the partition axis:
#   qT = wq_c.T @ x.T        [256, S]   (scale folded into wq_c)
#   kvT = wkv_c.T @ x.T      [128, S]   (k rows 0:64, v rows 64:128)
#   scoresT[j, i] = kT.T q   [128-block j, 512-chunk i]  (K=64)
#   causal mask: an extra matmul accumulates -80 * (1 - mask) into the
#   scores psum ((-80 I).T @ cminv), so exp() of masked entries ~ 1e-33
#   attn_T[d, i] = [v|1].T @ exp(scores)   (row 64 = softmax denoms)
#   out_partial[s, :] = attn_T.T-contracted with wo_c rows (fp16 out,
#   host accumulates)
#
# v2 structure: attention runs in two head-WAVES per 512-query chunk
# (wave w covers heads 2w, 2w+1).  A wave's two heads share one
# [128, 1024] 2-bank score-psum tile, so ONE wide (strided) exp per
# (J-block, wave) halves the Scalar-engine instruction count - the
# scalar exp stream is the attention-phase bottleneck.  PSUM is split
# into pool `pq` (4 x 2KB rotating: psq / po accumulators / o_proj /
# selector) and pool `sc` (2 x 4KB: kv-projection pair / score pairs).
# PV matmuls lag scores by 4 J-blocks; the softmax finalize is split in
# two (fin1: DVE reciprocal chain; fin2: selector matmul + normalize)
# emitted a few J-blocks apart so the PE never waits on the reciprocal.
# o_proj units of chunk ic are spread through chunk ic+1's J-loops as
# dense PE filler; the last chunk's units evacuate via the (then-idle)
# Scalar engine.

import sys
from contextlib import ExitStack

for _p in ("/opt/trn_rl_repo", "/root/.axon_site"):
    if _p not in sys.path:
        sys.path.insert(0, _p)

import numpy as np

import concourse.bacc as bacc
import concourse.mybir as mybir
import concourse.tile as tile
from concourse.bass_utils import run_bass_kernel_spmd

F32 = mybir.dt.float32
EXP = mybir.ActivationFunctionType.Exp

# matmul operand dtype: float32r (2 cyc/row, ~1.5e-4/matmul) or
# float16 (1 cyc/row, ~7e-4/matmul)
MM_DT = mybir.dt.float16
MASK_NEG = 80.0

S = 2048          # sequence length
H = 2048          # hidden size
NH = 32           # query heads
NKV = 8           # kv heads
HD = 64           # head dim
NCORES = 8
HPC = NH // NCORES        # query heads per core = 4
DQ = HPC * HD             # per-core q width = 256
SCALE = HD ** -0.5
P = 128
NB = S // P               # 16 128-blocks along seq
NC4 = S // 512            # 4 512-chunks along seq
KCH = H // P              # 16 contraction chunks
HS = S // 2               # half seq


def _build_module(mm_dt):
    nc = bacc.Bacc(trn_type="TRN2", debug=False)

    xT_d = nc.dram_tensor("xT", [H, S], mm_dt, kind="ExternalInput").ap()
    wq_d = nc.dram_tensor("wq", [P, KCH * DQ], mm_dt, kind="ExternalInput").ap()
    wkv_d = nc.dram_tensor("wkv", [P, KCH * P], mm_dt, kind="ExternalInput").ap()
    wo_d = nc.dram_tensor("wo", [P, 2 * S], mm_dt, kind="ExternalInput").ap()
    cos_d = nc.dram_tensor("cos2", [P, S], mm_dt, kind="ExternalInput").ap()
    sin_d = nc.dram_tensor("sin2", [P, S], mm_dt, kind="ExternalInput").ap()
    # inverted causal masks for the 4 diagonal offsets, and -80*I
    cm_d = nc.dram_tensor("cminv", [P, 4 * 512], mm_dt, kind="ExternalInput").ap()
    ni_d = nc.dram_tensor("negi", [P, P], mm_dt, kind="ExternalInput").ap()
    # selector matrix for the denominator broadcast (per wave)
    ew_d = nc.dram_tensor("ew", [P, P], mm_dt, kind="ExternalInput").ap()
    out_d = nc.dram_tensor("out", [S, H], mm_dt, kind="ExternalOutput").ap()

    with tile.TileContext(nc) as tc, ExitStack() as ctx:
        pers = ctx.enter_context(tc.tile_pool(name="pers", bufs=1))

        wq_sb = pers.tile([P, KCH * DQ], mm_dt, tag="wq_sb", name="wq_sb")
        wkv_sb = pers.tile([P, KCH * P], mm_dt, tag="wkv_sb", name="wkv_sb")
        cos_sb = pers.tile([P, S], mm_dt, tag="cos_sb", name="cos_sb")
        sin_sb = pers.tile([P, S], mm_dt, tag="sin_sb", name="sin_sb")
        wo_sb = pers.tile([P, 2 * S], mm_dt, tag="wo_sb", name="wo_sb")
        cm_sb = pers.tile([P, 4 * 512], mm_dt, tag="cm_sb", name="cm_sb")
        ni_sb = pers.tile([P, P], mm_dt, tag="ni_sb", name="ni_sb")
        ew_sb = pers.tile([P, P], mm_dt, tag="ew_sb", name="ew_sb")

        ones16 = pers.tile([P, NB], mm_dt, tag="ones16", name="ones16")
        nc.vector.memset(ones16[:], 1.0)

        # per-half persistent tensors (w == m: wave w covers heads 2w,2w+1)
        qrh = [[pers.tile([P, HS], mm_dt, tag=f"qr{m}_{g}", name=f"qr{m}_{g}")
                for g in range(2)] for m in range(2)]
        ktdh = [pers.tile([P, HS], mm_dt, tag=f"ktd{g}", name=f"ktd{g}")
                for g in range(2)]
        vtmph = [pers.tile([64, HS], mm_dt, tag=f"vtmp{g}", name=f"vtmp{g}")
                 for g in range(2)]
        vbh = [pers.tile([P, 8 * (HD + 1)], mm_dt, tag=f"vb{g}", name=f"vb{g}")
               for g in range(2)]
        attn0 = pers.tile([P, S], mm_dt, tag="attn0", name="attn0")
        attn1 = pers.tile([P, S], mm_dt, tag="attn1", name="attn1")
        attn = [attn0, attn1]
        rsum = pers.tile([P, 512], F32, tag="rsum", name="rsum")
        nc.vector.memset(rsum[:], 1.0)
        rrf = pers.tile([P, 512], F32, tag="rrf", name="rrf")
        rrs = pers.tile([P, 512], F32, tag="rrs", name="rrs")
        rr = pers.tile([P, 512], mm_dt, tag="rr", name="rr")

        for g in range(2):
            vbv = vbh[g][:].rearrange("p (b c) -> p b c", c=HD + 1)
            nc.vector.tensor_copy(vbv[:, :, HD:HD + 1], ones16[:, 0:8])

        # pools; PSUM: pq = 4 x 2KB banks, sc = 2 x 4KB (2-bank) slots
        xp = ctx.enter_context(tc.tile_pool(name="xp", bufs=6))
        rs = ctx.enter_context(tc.tile_pool(name="rs", bufs=2))
        pq = ctx.enter_context(tc.tile_pool(name="pq", bufs=4, space="PSUM"))
        scp = ctx.enter_context(tc.tile_pool(name="scp", bufs=2, space="PSUM"))
        pxp = ctx.enter_context(tc.tile_pool(name="pxp", bufs=7))
        otp = ctx.enter_context(tc.tile_pool(name="otp", bufs=3))

        psq_store = {}
        pskv_store = {}

        # ---------------- projections ----------------
        def proj_mm(g):
            nw = slice(1024 * g, 1024 * g + 1024)
            psq = [[pq.tile([P, 512], F32, tag="pq", name=f"psq{m}_{half}")
                    for half in range(2)] for m in range(2)]
            pskv = scp.tile([P, 1024], F32, tag="sc", name="pskv")
            psq_store[g] = psq
            pskv_store[g] = pskv
            for k in range(KCH):
                if g == 0 and (k in (0, 1) or (k % 4 == 2 and k < 12)):
                    # k=0/1 come alone so the first matmuls start early
                    if k == 0:
                        wqs, wks = slice(0, 256), slice(0, 128)
                    elif k == 1:
                        wqs, wks = slice(256, 1024), slice(128, 512)
                    else:
                        kg = k // 4 + 1
                        wqs = slice(1024 * kg, 1024 * kg + 1024)
                        wks = slice(512 * kg, 512 * kg + 512)
                    nc.sync.dma_start(wq_sb[:, wqs], wq_d[:, wqs])
                    nc.sync.dma_start(wkv_sb[:, wks], wkv_d[:, wks])
                xt = xp.tile([P, 1024], mm_dt, tag="xt", name="xt")
                if g == 0 and k < 2:
                    # split the first x tiles so the first matmul's data
                    # lands sooner
                    nc.sync.dma_start(
                        xt[:, 0:512], xT_d[P * k:P * k + P, 0:512])
                    nc.sync.dma_start(
                        xt[:, 512:1024], xT_d[P * k:P * k + P, 512:1024])
                else:
                    nc.sync.dma_start(xt[:], xT_d[P * k:P * k + P, nw])
                st = dict(start=(k == 0), stop=(k == KCH - 1))
                for half in range(2):
                    xs = xt[:, 512 * half:512 * half + 512]
                    nc.tensor.matmul(
                        psq[0][half][:], wq_sb[:, k * DQ:k * DQ + P], xs, **st)
                    nc.tensor.matmul(
                        psq[1][half][:], wq_sb[:, k * DQ + P:k * DQ + DQ],
                        xs, **st)
                    nc.tensor.matmul(
                        pskv[:, 512 * half:512 * half + 512],
                        wkv_sb[:, k * P:k * P + P], xs, **st)
                if g == 0 and k == 3:
                    # trig tables needed by the first RoPE below
                    nc.sync.dma_start(cos_sb[:], cos_d[:])
                    nc.sync.dma_start(sin_sb[:], sin_d[:])
                if g == 0 and k == 9:
                    nc.sync.dma_start(cm_sb[:], cm_d[:])
                    nc.sync.dma_start(ni_sb[:], ni_d[:])
                    nc.sync.dma_start(ew_sb[:], ew_d[:])
                    nc.sync.dma_start(wo_sb[:], wo_d[:])

        # ---------------- RoPE ----------------
        def rope(g):
            # PSUM evacuation is split across Scalar and DVE (fp16
            # casts); for g=0 the q casts go first (the psq slots gate
            # the g=1 projection), for g=1 the kv casts go on DVE (they
            # gate the chunk-0 score tiles and must not queue behind the
            # chunk-0 exps on the scalar engine).  Then an SBUF-only
            # fp16 rotate-half chain on DVE.
            nw = slice(1024 * g, 1024 * g + 1024)
            psq = psq_store.pop(g)
            pskv = pskv_store.pop(g)
            q16s = []
            for m in range(2):
                q16 = rs.tile([P, 1024], mm_dt, tag=f"q16_{m}", name=f"q16_{m}")
                q16s.append(q16)
                eng = nc.vector if (g == 0 and m == 0) else nc.scalar
                for half in range(2):
                    hs = slice(512 * half, 512 * half + 512)
                    if eng is nc.vector:
                        nc.vector.tensor_copy(q16[:, hs], psq[m][half][:])
                    else:
                        nc.scalar.copy(q16[:, hs], psq[m][half][:])
            k16 = rs.tile([64, 1024], mm_dt, tag="k16", name="k16")
            for half in range(2):
                hs = slice(512 * half, 512 * half + 512)
                if g == 1:
                    nc.vector.tensor_copy(k16[:, hs], pskv[0:64, hs])
                else:
                    nc.scalar.copy(k16[:, hs], pskv[0:64, hs])
            for half in range(2):
                hs = slice(512 * half, 512 * half + 512)
                if g == 1:
                    nc.vector.tensor_copy(vtmph[g][:, hs], pskv[64:P, hs])
                else:
                    nc.scalar.copy(vtmph[g][:, hs], pskv[64:P, hs])
            # fp16 SBUF chains on DVE
            for m in range(2):
                q16 = q16s[m]
                nc.vector.tensor_mul(qrh[m][g][:], q16[:], cos_sb[:, nw])
                qsw = rs.tile([P, 1024], mm_dt, tag=f"qsw{m}", name=f"qsw{m}")
                for b0 in (0, 64):
                    nc.vector.tensor_copy(
                        qsw[b0:b0 + 32, :], q16[b0 + 32:b0 + 64, :])
                    nc.vector.tensor_copy(
                        qsw[b0 + 32:b0 + 64, :], q16[b0:b0 + 32, :])
                nc.vector.tensor_mul(qsw[:], qsw[:], sin_sb[:, nw])
                nc.vector.tensor_add(qrh[m][g][:], qrh[m][g][:], qsw[:])
            nc.vector.tensor_mul(ktdh[g][0:64, :], k16[:], cos_sb[0:64, nw])
            ksw = rs.tile([64, 1024], mm_dt, tag="ksw", name="ksw")
            nc.vector.tensor_copy(ksw[0:32, :], k16[32:64, :])
            nc.vector.tensor_copy(ksw[32:64, :], k16[0:32, :])
            nc.vector.tensor_mul(ksw[:], ksw[:], sin_sb[0:64, nw])
            nc.vector.tensor_add(ktdh[g][0:64, :], ktdh[g][0:64, :], ksw[:])
            # duplicate k rows for the upper-head score matmuls
            nc.vector.tensor_copy(ktdh[g][64:P, :], ktdh[g][0:64, :])
            # stream-transpose v into vb blocks (DVE)
            vbv = vbh[g][:].rearrange("p (b c) -> p b c", c=HD + 1)
            vtv = vtmph[g][:].rearrange("p (b c) -> p b c", c=P)
            for a in range(4):
                for b in range(2):
                    nc.vector.transpose(
                        vbv[32 * a:32 * a + 32, :, 32 * b:32 * b + 32],
                        vtv[32 * b:32 * b + 32, :, 32 * a:32 * a + 32])

        # ---------------- attention ----------------
        pxstore = {}

        def emit_scores(ic, J, w):
            icg, icr = ic // 2, ic % 2
            t = J - 4 * ic
            c0 = 128 * t if t > 0 else 0
            Jg, Jr = J // 8, J % 8
            Js = slice(P * Jr, P * Jr + P)
            qs = slice(512 * icr + c0, 512 * icr + 512)
            ps_s = scp.tile([P, 1024], F32, tag="sc", name="ps_s")
            for hh in range(2):
                b0, col = 64 * hh, 512 * hh
                nc.tensor.matmul(
                    ps_s[:, col + c0:col + 512], ktdh[Jg][b0:b0 + 64, Js],
                    qrh[w][icg][b0:b0 + 64, qs],
                    start=True, stop=(t < 0))
            if t >= 0:
                for hh in range(2):
                    col = 512 * hh
                    nc.tensor.matmul(
                        ps_s[:, col + c0:col + 512], ni_sb[:],
                        cm_sb[:, 512 * t + c0:512 * t + 512],
                        start=False, stop=True)
            px = pxp.tile([P, 1024], mm_dt, tag="pxp", name="px")
            # one strided exp covering both heads' live regions
            ps_v = ps_s[:].rearrange("p (two c) -> p two c", two=2)
            px_v = px[:].rearrange("p (two c) -> p two c", two=2)
            nc.scalar.activation(px_v[:, :, c0:], ps_v[:, :, c0:], EXP)
            pxstore[(ic, J, w)] = px

        def emit_pv(ic, J, w, po_w):
            t = J - 4 * ic
            c0 = 128 * t if t > 0 else 0
            nJ = 4 * ic + 4
            Jg, Jr = J // 8, J % 8
            vs = slice((HD + 1) * Jr, (HD + 1) * Jr + HD + 1)
            px = pxstore.pop((ic, J, w))
            for hh in range(2):
                col = 512 * hh
                nc.tensor.matmul(
                    po_w[hh][:, c0:], vbh[Jg][:, vs], px[:, col + c0:col + 512],
                    start=(J == 0), stop=(J == nJ - 1))

        def fin1(ic, w, po_w):
            # DVE-only: gather the 2 denominator rows, batched reciprocal
            # (fast variant: ~18 correct bits, far above the fp16 rr cast)
            for hh in range(2):
                nc.vector.tensor_copy(
                    rsum[32 * hh:32 * hh + 1, :], po_w[hh][HD:HD + 1, :])
            with nc.allow_low_precision(reason="softmax reciprocal"):
                nc.vector.reciprocal_approx_fast(rrf[:], rsum[:])
                nc.vector.tensor_copy(rr[:], rrf[:])

        def fin2(ic, w, po_w):
            # selector matmul broadcasts the reciprocals, then normalize
            psb = pq.tile([P, 512], F32, tag="pq", name="psb")
            nc.tensor.matmul(psb[:], ew_sb[:], rr[:], start=True, stop=True)
            psbs = rs.tile([P, 512], F32, tag="psbs", name="psbs")
            nc.vector.tensor_copy(psbs[:], psb[:])
            for hh in range(2):
                b0 = 64 * hh
                asl = attn[w][b0:b0 + 64, 512 * ic:512 * ic + 512]
                nc.vector.tensor_mul(asl, po_w[hh][0:HD, :], psbs[b0:b0 + 64, :])

        ot_store = {}

        def emit_oproj_unit(sb, n4, tail=False):
            ss = slice(P * sb, P * sb + P)
            ps_o = pq.tile([P, 512], F32, tag="pq", name="ps_o")
            nc.tensor.matmul(
                ps_o[:], attn0[:, ss],
                wo_sb[:, 512 * n4:512 * n4 + 512],
                start=True, stop=False)
            nc.tensor.matmul(
                ps_o[:], attn1[:, ss],
                wo_sb[:, S + 512 * n4:S + 512 * n4 + 512],
                start=False, stop=True)
            if n4 == 0:
                ot_store[sb] = otp.tile([P, H], mm_dt, tag="otp", name="ot")
            ot = ot_store[sb]
            osl = slice(512 * n4, 512 * n4 + 512)
            if tail:
                # both scalar and DVE are idle at the tail: alternate the
                # casts and DMA per unit so transfers overlap them
                if n4 % 2 == 0:
                    nc.scalar.copy(ot[:, osl], ps_o[:])
                else:
                    nc.vector.tensor_copy(ot[:, osl], ps_o[:])
                nc.sync.dma_start(out_d[ss, osl], ot[:, osl])
                if n4 == NC4 - 1:
                    ot_store.pop(sb)
            else:
                nc.vector.tensor_copy(ot[:, osl], ps_o[:])
                if n4 == NC4 - 1:
                    nc.sync.dma_start(out_d[ss, :], ot_store.pop(sb)[:])

        # ---------------- emission ----------------
        proj_mm(0)
        rope(0)
        proj_mm(1)
        # chunk-0's first score groups (and their exps) are emitted before
        # rope(1) so the scalar engine starts the exp stream immediately
        # after the g=1 projection instead of behind rope(1)'s casts
        emit_scores(0, 0, 0)
        emit_scores(0, 1, 0)
        rope(1)
        pending_fin2 = None
        units = []
        ui = si = 0
        slots_total = 1

        for ic in range(NC4):
            nJ = 4 * ic + 4
            # filler units: o_proj of the previous chunk, spread over this
            # chunk's score/PV loop iterations (skipping the first two of
            # each wave, which cover fin2 / chunk-boundary latency)
            units = [(sb, n4) for sb in range(4 * (ic - 1), 4 * ic)
                     for n4 in range(NC4)] if ic > 0 else []
            ui = si = 0
            slots_total = max(1, 2 * (nJ - 5))
            for w in range(2):
                po_w = None
                npv = 0
                iters = list(range(2, nJ))
                for idx, J in enumerate(iters):
                    emit_scores(ic, J, w)
                    if idx == 1 and pending_fin2 is not None:
                        fin2(*pending_fin2)
                        pending_fin2 = None
                    if idx >= 2:
                        if po_w is None:
                            po_w = [pq.tile([HD + 1, 512], F32, tag="pq",
                                            name=f"po{ic}_{w}_{hh}")
                                    for hh in range(2)]
                        emit_pv(ic, npv, w, po_w)
                        npv += 1
                        # fillers skip the wave's last iteration so the
                        # DVE is free for the softmax-finalize chain
                        if idx < len(iters) - 1:
                            si += 1
                            take = (len(units) * si) // slots_total - ui
                            while take > 0 and ui < len(units):
                                emit_oproj_unit(*units[ui])
                                ui += 1
                                take -= 1
                if po_w is None:
                    po_w = [pq.tile([HD + 1, 512], F32, tag="pq",
                                    name=f"po{ic}_{w}_{hh}")
                            for hh in range(2)]
                while npv < nJ:
                    emit_pv(ic, npv, w, po_w)
                    npv += 1
                fin1(ic, w, po_w)
                # lookahead: first two score groups of the next wave/chunk
                if w == 0:
                    emit_scores(ic, 0, 1)
                    emit_scores(ic, 1, 1)
                elif ic + 1 < NC4:
                    emit_scores(ic + 1, 0, 0)
                    emit_scores(ic + 1, 1, 0)
                pending_fin2 = (ic, w, po_w)
            while ui < len(units):
                emit_oproj_unit(*units[ui])
                ui += 1
        # tail: finalize the last wave, then its o_proj via scalar casts
        fin2(*pending_fin2)
        for sb in range(12, 16):
            for n4 in range(NC4):
                emit_oproj_unit(sb, n4, tail=True)

    nc.compile()
    return nc


_NC_CACHE = {}


def _get_module(mm_dt=MM_DT):
    if mm_dt not in _NC_CACHE:
        _NC_CACHE[mm_dt] = _build_module(mm_dt)
    return _NC_CACHE[mm_dt]


def _prep_inputs(x, wq, wk, wv, wo, cos, sin, mm_dt=MM_DT):
    mm_np = mybir.dt.np(mm_dt)
    x = np.asarray(x, dtype=np.float32)
    xT = np.ascontiguousarray(x.reshape(S, H).T.astype(mm_np))

    cosT = np.asarray(cos, dtype=np.float32).T          # [64, S]
    sinT = np.asarray(sin, dtype=np.float32).T          # [64, S]
    sgn = np.where(np.arange(HD) < HD // 2, -1.0, 1.0).astype(np.float32)
    sinT_s = sinT * sgn[:, None]
    cos2 = np.ascontiguousarray(np.tile(cosT, (2, 1))).astype(mm_np)  # [128, S]
    sin2 = np.ascontiguousarray(np.tile(sinT_s, (2, 1))).astype(mm_np)

    # inverted causal masks (1 where masked out), diagonal offsets 0..3
    jl = np.arange(P)[:, None]
    il = np.arange(512)[None, :]
    cminv = np.concatenate(
        [(jl + P * t > il).astype(np.float32) for t in range(4)], axis=1)
    cminv = np.ascontiguousarray(cminv).astype(mm_np)
    negi = (-MASK_NEG * np.eye(P, dtype=np.float32)).astype(mm_np)

    # selector matrix: psb rows 0:64 get the reciprocal row of the wave's
    # first head (partition 0), rows 64:128 the second head (partition 32)
    ew = np.zeros((P, P), dtype=np.float32)
    ew[0, 0:64] = 1.0
    ew[32, 64:128] = 1.0
    ew = ew.astype(mm_np)

    def chunk_kxm(w):
        # [H, M] -> [128, KCH*M] with k-chunk-major free layout
        m = w.shape[1]
        return np.ascontiguousarray(
            w.reshape(KCH, P, m).transpose(1, 0, 2).reshape(P, KCH * m).astype(mm_np))

    wq = np.asarray(wq, dtype=np.float32)
    wk = np.asarray(wk, dtype=np.float32)
    wv = np.asarray(wv, dtype=np.float32)
    wo = np.asarray(wo, dtype=np.float32)

    in_maps = []
    for c in range(NCORES):
        wq_c = wq[:, DQ * c:DQ * c + DQ] * SCALE
        wkv_c = np.concatenate(
            [wk[:, HD * c:HD * c + HD], wv[:, HD * c:HD * c + HD]], axis=1)
        wo_c = wo[DQ * c:DQ * c + DQ, :]
        wo_l = np.ascontiguousarray(
            wo_c.reshape(2, P, H).transpose(1, 0, 2).reshape(P, 2 * H).astype(mm_np))
        in_maps.append({
            "xT": xT,
            "wq": chunk_kxm(wq_c),
            "wkv": chunk_kxm(wkv_c),
            "wo": wo_l,
            "cos2": cos2,
            "sin2": sin2,
            "cminv": cminv,
            "negi": negi,
            "ew": ew,
        })
    return in_maps


def run(inputs, trace=False, trace_kwargs=None, mm_dt=MM_DT):
    """Execute on 8 cores; returns (full_output, BassKernelResults)."""
    nc = _get_module(mm_dt)
    in_maps = _prep_inputs(
        inputs["x"], inputs["wq"], inputs["wk"], inputs["wv"],
        inputs["wo"], inputs["cos"], inputs["sin"], mm_dt=mm_dt)
    kwargs = {}
    if trace:
        kwargs = dict(trace=True, **(trace_kwargs or {}))
    res = run_bass_kernel_spmd(nc, in_maps, core_ids=list(range(NCORES)), **kwargs)
    acc = np.zeros((S, H), dtype=np.float32)
    for c in range(NCORES):
        acc += res.results[c]["out"].astype(np.float32)
    out = acc.reshape(1, S, H)
    return out, res


def kernel(**inputs):
    out, _ = run(inputs, trace=False)
    return out


# revision 21
# speedup vs baseline: 1.2193x; 1.0325x over previous
# Tensor-parallel GQA attention kernel for 8 Trainium2 NeuronCores.
#
# Sharding: each core owns 4 query heads + 1 kv head (32 q / 8 kv heads
# total), computes q/k/v projections for its heads, RoPE, causal
# attention, and a partial o_proj (row slice of wo); the host sums the 8
# partial outputs.
#
# Per-core layout: everything is kept "transposed" ([dim, seq]) so the
# contraction dim of every matmul is the partition axis:
#   qT = wq_c.T @ x.T        [256, S]   (scale folded into wq_c)
#   kvT = wkv_c.T @ x.T      [128, S]   (k rows 0:64, v rows 64:128)
#   scoresT[j, i] = kT.T q   [128-block j, 512-chunk i]  (K=64)
#   causal mask: an extra matmul accumulates -80 * (1 - mask) into the
#   scores psum ((-80 I).T @ cminv), so exp() of masked entries ~ 1e-33
#   attn_T[d, i] = [v|1].T @ exp(scores)   (row 64 = softmax denoms)
#   out_partial[s, :] = attn_T.T-contracted with wo_c rows (fp16 out,
#   host accumulates)
#
# v2 structure: attention runs in two head-WAVES per 512-query chunk
# (wave w covers heads 2w, 2w+1).  A wave's two heads share one
# [128, 1024] 2-bank score-psum tile, so ONE wide (strided) exp per
# (J-block, wave) halves the Scalar-engine instruction count - the
# scalar exp stream is the attention-phase bottleneck.  PSUM is split
# into pool `pq` (4 x 2KB rotating: psq / po accumulators / o_proj /
# selector) and pool `sc` (2 x 4KB: kv-projection pair / score pairs).
# PV matmuls lag scores by 4 J-blocks; the softmax finalize is split in
# two (fin1: DVE reciprocal chain; fin2: selector matmul + normalize)
# emitted a few J-blocks apart so the PE never waits on the reciprocal.
# o_proj units of chunk ic are spread through chunk ic+1's J-loops as
# dense PE filler; the last chunk's units evacuate via the (then-idle)
# Scalar engine.

import sys
from contextlib import ExitStack

for _p in ("/opt/trn_rl_repo", "/root/.axon_site"):
    if _p not in sys.path:
        sys.path.insert(0, _p)

import numpy as np

import concourse.bacc as bacc
import concourse.mybir as mybir
import concourse.tile as tile
from concourse.bass_utils import run_bass_kernel_spmd

F32 = mybir.dt.float32
EXP = mybir.ActivationFunctionType.Exp

# matmul operand dtype: float32r (2 cyc/row, ~1.5e-4/matmul) or
# float16 (1 cyc/row, ~7e-4/matmul)
MM_DT = mybir.dt.float16
MASK_NEG = 80.0

S = 2048          # sequence length
H = 2048          # hidden size
NH = 32           # query heads
NKV = 8           # kv heads
HD = 64           # head dim
NCORES = 8
HPC = NH // NCORES        # query heads per core = 4
DQ = HPC * HD             # per-core q width = 256
SCALE = HD ** -0.5
P = 128
NB = S // P               # 16 128-blocks along seq
NC4 = S // 512            # 4 512-chunks along seq
KCH = H // P              # 16 contraction chunks
HS = S // 2               # half seq


def _build_module(mm_dt):
    nc = bacc.Bacc(trn_type="TRN2", debug=False)

    xT_d = nc.dram_tensor("xT", [H, S], mm_dt, kind="ExternalInput").ap()
    wq_d = nc.dram_tensor("wq", [P, KCH * DQ], mm_dt, kind="ExternalInput").ap()
    wkv_d = nc.dram_tensor("wkv", [P, KCH * P], mm_dt, kind="ExternalInput").ap()
    wo_d = nc.dram_tensor("wo", [P, 2 * S], mm_dt, kind="ExternalInput").ap()
    cos_d = nc.dram_tensor("cos2", [P, S], mm_dt, kind="ExternalInput").ap()
    sin_d = nc.dram_tensor("sin2", [P, S], mm_dt, kind="ExternalInput").ap()
    # inverted causal masks for the 4 diagonal offsets, and -80*I
    cm_d = nc.dram_tensor("cminv", [P, 4 * 512], mm_dt, kind="ExternalInput").ap()
    ni_d = nc.dram_tensor("negi", [P, P], mm_dt, kind="ExternalInput").ap()
    # selector matrix for the denominator broadcast (per wave)
    ew_d = nc.dram_tensor("ew", [P, P], mm_dt, kind="ExternalInput").ap()
    out_d = nc.dram_tensor("out", [S, H], mm_dt, kind="ExternalOutput").ap()

    with tile.TileContext(nc) as tc, ExitStack() as ctx:
        pers = ctx.enter_context(tc.tile_pool(name="pers", bufs=1))

        wq_sb = pers.tile([P, KCH * DQ], mm_dt, tag="wq_sb", name="wq_sb")
        wkv_sb = pers.tile([P, KCH * P], mm_dt, tag="wkv_sb", name="wkv_sb")
        cos_sb = pers.tile([P, S], mm_dt, tag="cos_sb", name="cos_sb")
        sin_sb = pers.tile([P, S], mm_dt, tag="sin_sb", name="sin_sb")
        wo_sb = pers.tile([P, 2 * S], mm_dt, tag="wo_sb", name="wo_sb")
        cm_sb = pers.tile([P, 4 * 512], mm_dt, tag="cm_sb", name="cm_sb")
        ni_sb = pers.tile([P, P], mm_dt, tag="ni_sb", name="ni_sb")
        ew_sb = pers.tile([P, P], mm_dt, tag="ew_sb", name="ew_sb")

        ones16 = pers.tile([P, NB], mm_dt, tag="ones16", name="ones16")
        nc.vector.memset(ones16[:], 1.0)

        # per-half persistent tensors (w == m: wave w covers heads 2w,2w+1)
        qrh = [[pers.tile([P, HS], mm_dt, tag=f"qr{m}_{g}", name=f"qr{m}_{g}")
                for g in range(2)] for m in range(2)]
        ktdh = [pers.tile([P, HS], mm_dt, tag=f"ktd{g}", name=f"ktd{g}")
                for g in range(2)]
        vtmph = [pers.tile([64, HS], mm_dt, tag=f"vtmp{g}", name=f"vtmp{g}")
                 for g in range(2)]
        vbh = [pers.tile([P, 8 * (HD + 1)], mm_dt, tag=f"vb{g}", name=f"vb{g}")
               for g in range(2)]
        attn0 = pers.tile([P, S], mm_dt, tag="attn0", name="attn0")
        attn1 = pers.tile([P, S], mm_dt, tag="attn1", name="attn1")
        attn = [attn0, attn1]
        rsum = pers.tile([P, 512], F32, tag="rsum", name="rsum")
        nc.vector.memset(rsum[:], 1.0)
        rrf = pers.tile([P, 512], F32, tag="rrf", name="rrf")
        rrs = pers.tile([P, 512], F32, tag="rrs", name="rrs")
        rr = pers.tile([P, 512], mm_dt, tag="rr", name="rr")

        for g in range(2):
            vbv = vbh[g][:].rearrange("p (b c) -> p b c", c=HD + 1)
            nc.vector.tensor_copy(vbv[:, :, HD:HD + 1], ones16[:, 0:8])

        # pools; PSUM: pq = 4 x 2KB banks, sc = 2 x 4KB (2-bank) slots
        xp = ctx.enter_context(tc.tile_pool(name="xp", bufs=18))
        rs = ctx.enter_context(tc.tile_pool(name="rs", bufs=2))
        pq = ctx.enter_context(tc.tile_pool(name="pq", bufs=4, space="PSUM"))
        scp = ctx.enter_context(tc.tile_pool(name="scp", bufs=2, space="PSUM"))
        pxp = ctx.enter_context(tc.tile_pool(name="pxp", bufs=9))
        otp = ctx.enter_context(tc.tile_pool(name="otp", bufs=3))

        psq_store = {}
        pskv_store = {}

        # ---------------- projections ----------------
        def proj_mm(g):
            # two passes over the k-chunks: q-matmuls and kv-matmuls are
            # split so the psum handoff to RoPE/the next phase overlaps
            # with the other projection's matmuls (g=0: q first, so the
            # psq banks release during the kv pass; g=1: kv first, so the
            # g=0 rope casts finish during the kv pass)
            nw = slice(1024 * g, 1024 * g + 1024)
            psq = [[pq.tile([P, 512], F32, tag="pq", name=f"psq{m}_{half}")
                    for half in range(2)] for m in range(2)]
            pskv = scp.tile([P, 1024], F32, tag="sc", name="pskv")
            psq_store[g] = psq
            pskv_store[g] = pskv

            def emit_q(k, xt):
                st = dict(start=(k == 0), stop=(k == KCH - 1))
                for half in range(2):
                    xs = xt[:, 512 * half:512 * half + 512]
                    nc.tensor.matmul(
                        psq[0][half][:], wq_sb[:, k * DQ:k * DQ + P], xs, **st)
                    nc.tensor.matmul(
                        psq[1][half][:], wq_sb[:, k * DQ + P:k * DQ + DQ],
                        xs, **st)

            def emit_kv(k, xt):
                st = dict(start=(k == 0), stop=(k == KCH - 1))
                for half in range(2):
                    xs = xt[:, 512 * half:512 * half + 512]
                    nc.tensor.matmul(
                        pskv[:, 512 * half:512 * half + 512],
                        wkv_sb[:, k * P:k * P + P], xs, **st)

            xts = []
            for k in range(KCH):
                if g == 0 and (k in (0, 1) or (k % 4 == 2 and k < 12)):
                    # k=0/1 come alone so the first matmuls start early
                    if k == 0:
                        wqs, wks = slice(0, 256), slice(0, 128)
                    elif k == 1:
                        wqs, wks = slice(256, 1024), slice(128, 512)
                    else:
                        kg = k // 4 + 1
                        wqs = slice(1024 * kg, 1024 * kg + 1024)
                        wks = slice(512 * kg, 512 * kg + 512)
                    nc.sync.dma_start(wq_sb[:, wqs], wq_d[:, wqs])
                    nc.sync.dma_start(wkv_sb[:, wks], wkv_d[:, wks])
                xt = xp.tile([P, 1024], mm_dt, tag="xt", name="xt")
                if g == 0 and k < 2:
                    # split the first x tiles so the first matmul's data
                    # lands sooner
                    nc.sync.dma_start(
                        xt[:, 0:512], xT_d[P * k:P * k + P, 0:512])
                    nc.sync.dma_start(
                        xt[:, 512:1024], xT_d[P * k:P * k + P, 512:1024])
                else:
                    nc.sync.dma_start(xt[:], xT_d[P * k:P * k + P, nw])
                xts.append(xt)
                if g == 0:
                    emit_q(k, xt)
                else:
                    emit_kv(k, xt)
                if g == 0 and k == 3:
                    # trig tables needed by the first RoPE below
                    nc.sync.dma_start(cos_sb[:], cos_d[:])
                    nc.sync.dma_start(sin_sb[:], sin_d[:])
                if g == 0 and k == 9:
                    nc.sync.dma_start(cm_sb[:], cm_d[:])
                    nc.sync.dma_start(ni_sb[:], ni_d[:])
                    nc.sync.dma_start(ew_sb[:], ew_d[:])
                    nc.sync.dma_start(wo_sb[:], wo_d[:])
            for k in range(KCH):
                if g == 0:
                    emit_kv(k, xts[k])
                else:
                    emit_q(k, xts[k])

        # ---------------- RoPE ----------------
        def rope(g):
            # PSUM evacuation is split across Scalar and DVE (fp16
            # casts); for g=0 the q casts go first (the psq slots gate
            # the g=1 projection), for g=1 the kv casts go on DVE (they
            # gate the chunk-0 score tiles and must not queue behind the
            # chunk-0 exps on the scalar engine).  Then an SBUF-only
            # fp16 rotate-half chain on DVE.
            nw = slice(1024 * g, 1024 * g + 1024)
            psq = psq_store.pop(g)
            pskv = pskv_store.pop(g)
            q16s = []
            for m in range(2):
                q16 = rs.tile([P, 1024], mm_dt, tag=f"q16_{m}", name=f"q16_{m}")
                q16s.append(q16)
                eng = nc.vector if (g == 0 and m == 0) else nc.scalar
                for half in range(2):
                    hs = slice(512 * half, 512 * half + 512)
                    if eng is nc.vector:
                        nc.vector.tensor_copy(q16[:, hs], psq[m][half][:])
                    else:
                        nc.scalar.copy(q16[:, hs], psq[m][half][:])
            k16 = rs.tile([64, 1024], mm_dt, tag="k16", name="k16")
            for half in range(2):
                hs = slice(512 * half, 512 * half + 512)
                if g == 1:
                    nc.vector.tensor_copy(k16[:, hs], pskv[0:64, hs])
                else:
                    nc.scalar.copy(k16[:, hs], pskv[0:64, hs])
            for half in range(2):
                hs = slice(512 * half, 512 * half + 512)
                if g == 1:
                    nc.vector.tensor_copy(vtmph[g][:, hs], pskv[64:P, hs])
                else:
                    nc.scalar.copy(vtmph[g][:, hs], pskv[64:P, hs])
            # fp16 SBUF chains on DVE
            for m in range(2):
                q16 = q16s[m]
                nc.vector.tensor_mul(qrh[m][g][:], q16[:], cos_sb[:, nw])
                qsw = rs.tile([P, 1024], mm_dt, tag=f"qsw{m}", name=f"qsw{m}")
                for b0 in (0, 64):
                    nc.vector.tensor_copy(
                        qsw[b0:b0 + 32, :], q16[b0 + 32:b0 + 64, :])
                    nc.vector.tensor_copy(
                        qsw[b0 + 32:b0 + 64, :], q16[b0:b0 + 32, :])
                nc.vector.tensor_mul(qsw[:], qsw[:], sin_sb[:, nw])
                nc.vector.tensor_add(qrh[m][g][:], qrh[m][g][:], qsw[:])
            nc.vector.tensor_mul(ktdh[g][0:64, :], k16[:], cos_sb[0:64, nw])
            ksw = rs.tile([64, 1024], mm_dt, tag="ksw", name="ksw")
            nc.vector.tensor_copy(ksw[0:32, :], k16[32:64, :])
            nc.vector.tensor_copy(ksw[32:64, :], k16[0:32, :])
            nc.vector.tensor_mul(ksw[:], ksw[:], sin_sb[0:64, nw])
            nc.vector.tensor_add(ktdh[g][0:64, :], ktdh[g][0:64, :], ksw[:])
            # duplicate k rows for the upper-head score matmuls
            nc.vector.tensor_copy(ktdh[g][64:P, :], ktdh[g][0:64, :])
            # stream-transpose v into vb blocks (DVE)
            vbv = vbh[g][:].rearrange("p (b c) -> p b c", c=HD + 1)
            vtv = vtmph[g][:].rearrange("p (b c) -> p b c", c=P)
            for a in range(4):
                for b in range(2):
                    nc.vector.transpose(
                        vbv[32 * a:32 * a + 32, :, 32 * b:32 * b + 32],
                        vtv[32 * b:32 * b + 32, :, 32 * a:32 * a + 32])

        # ---------------- attention ----------------
        pxstore = {}

        def emit_scores(ic, J, w):
            icg, icr = ic // 2, ic % 2
            t = J - 4 * ic
            c0 = 128 * t if t > 0 else 0
            Jg, Jr = J // 8, J % 8
            Js = slice(P * Jr, P * Jr + P)
            qs = slice(512 * icr + c0, 512 * icr + 512)
            ps_s = scp.tile([P, 1024], F32, tag="sc", name="ps_s")
            for hh in range(2):
                b0, col = 64 * hh, 512 * hh
                nc.tensor.matmul(
                    ps_s[:, col + c0:col + 512], ktdh[Jg][b0:b0 + 64, Js],
                    qrh[w][icg][b0:b0 + 64, qs],
                    start=True, stop=(t < 0))
            if t >= 0:
                for hh in range(2):
                    col = 512 * hh
                    nc.tensor.matmul(
                        ps_s[:, col + c0:col + 512], ni_sb[:],
                        cm_sb[:, 512 * t + c0:512 * t + 512],
                        start=False, stop=True)
            px = pxp.tile([P, 1024], mm_dt, tag="pxp", name="px")
            # one strided exp covering both heads' live regions
            ps_v = ps_s[:].rearrange("p (two c) -> p two c", two=2)
            px_v = px[:].rearrange("p (two c) -> p two c", two=2)
            nc.scalar.activation(px_v[:, :, c0:], ps_v[:, :, c0:], EXP)
            pxstore[(ic, J, w)] = px

        def emit_pv(ic, J, w, po_w):
            t = J - 4 * ic
            c0 = 128 * t if t > 0 else 0
            nJ = 4 * ic + 4
            Jg, Jr = J // 8, J % 8
            vs = slice((HD + 1) * Jr, (HD + 1) * Jr + HD + 1)
            px = pxstore.pop((ic, J, w))
            for hh in range(2):
                col = 512 * hh
                nc.tensor.matmul(
                    po_w[hh][:, c0:], vbh[Jg][:, vs], px[:, col + c0:col + 512],
                    start=(J == 0), stop=(J == nJ - 1))

        def fin1(ic, w, po_w):
            # DVE-only: gather the 2 denominator rows, batched reciprocal
            # (fast variant: ~18 correct bits, far above the fp16 rr cast)
            for hh in range(2):
                nc.vector.tensor_copy(
                    rsum[32 * hh:32 * hh + 1, :], po_w[hh][HD:HD + 1, :])
            with nc.allow_low_precision(reason="softmax reciprocal"):
                nc.vector.reciprocal_approx_fast(rrf[:], rsum[:])
                nc.vector.tensor_copy(rr[:], rrf[:])

        def fin2(ic, w, po_w):
            # selector matmul broadcasts the reciprocals, then normalize
            psb = pq.tile([P, 512], F32, tag="pq", name="psb")
            nc.tensor.matmul(psb[:], ew_sb[:], rr[:], start=True, stop=True)
            psbs = rs.tile([P, 512], F32, tag="psbs", name="psbs")
            nc.vector.tensor_copy(psbs[:], psb[:])
            for hh in range(2):
                b0 = 64 * hh
                asl = attn[w][b0:b0 + 64, 512 * ic:512 * ic + 512]
                nc.vector.tensor_mul(asl, po_w[hh][0:HD, :], psbs[b0:b0 + 64, :])

        ot_store = {}

        def emit_oproj_unit(sb, n4, tail=False):
            ss = slice(P * sb, P * sb + P)
            ps_o = pq.tile([P, 512], F32, tag="pq", name="ps_o")
            nc.tensor.matmul(
                ps_o[:], attn0[:, ss],
                wo_sb[:, 512 * n4:512 * n4 + 512],
                start=True, stop=False)
            nc.tensor.matmul(
                ps_o[:], attn1[:, ss],
                wo_sb[:, S + 512 * n4:S + 512 * n4 + 512],
                start=False, stop=True)
            if n4 == 0:
                ot_store[sb] = otp.tile([P, H], mm_dt, tag="otp", name="ot")
            ot = ot_store[sb]
            osl = slice(512 * n4, 512 * n4 + 512)
            if tail:
                # both scalar and DVE are idle at the tail: alternate casts
                if n4 % 2 == 0:
                    nc.scalar.copy(ot[:, osl], ps_o[:])
                else:
                    nc.vector.tensor_copy(ot[:, osl], ps_o[:])
            else:
                nc.vector.tensor_copy(ot[:, osl], ps_o[:])
            if n4 == NC4 - 1:
                # trigger output DMA from the (idle) gpsimd queue - sync
                # queue triggers cost ~600ns each and serialize the tail
                nc.gpsimd.dma_start(out_d[ss, :], ot_store.pop(sb)[:])

        # ---------------- emission ----------------
        proj_mm(0)
        rope(0)
        proj_mm(1)
        # chunk-0's first score groups (and their exps) are emitted before
        # rope(1) so the scalar engine starts the exp stream immediately
        # after the g=1 projection instead of behind rope(1)'s casts
        emit_scores(0, 0, 0)
        emit_scores(0, 1, 0)
        rope(1)
        pending_fin2 = None
        units = []
        ui = si = 0
        slots_total = 1

        for ic in range(NC4):
            nJ = 4 * ic + 4
            # filler units: o_proj of the previous chunk, spread over this
            # chunk's score/PV loop iterations (skipping the first two of
            # each wave, which cover fin2 / chunk-boundary latency)
            units = [(sb, n4) for sb in range(4 * (ic - 1), 4 * ic)
                     for n4 in range(NC4)] if ic > 0 else []
            ui = si = 0
            slots_total = max(1, 2 * (nJ - 5))
            for w in range(2):
                po_w = None
                npv = 0
                iters = list(range(2, nJ))
                # lookahead target: first two score groups of the next
                # wave/chunk, emitted mid-loop (long waves) so their exps
                # are already drained when the next wave's PVs need them
                if w == 0:
                    nxt = (ic, 1)
                elif ic + 1 < NC4:
                    nxt = (ic + 1, 0)
                else:
                    nxt = None
                inloop_la = len(iters) >= 6
                for idx, J in enumerate(iters):
                    emit_scores(ic, J, w)
                    if idx == 1 and pending_fin2 is not None:
                        fin2(*pending_fin2)
                        pending_fin2 = None
                    if idx >= 2:
                        if po_w is None:
                            po_w = [pq.tile([HD + 1, 512], F32, tag="pq",
                                            name=f"po{ic}_{w}_{hh}")
                                    for hh in range(2)]
                        emit_pv(ic, npv, w, po_w)
                        npv += 1
                        if inloop_la and nxt is not None and \
                                idx in (len(iters) - 4, len(iters) - 3):
                            emit_scores(nxt[0], idx - (len(iters) - 4), nxt[1])
                        # fillers skip the wave's last iteration so the
                        # DVE is free for the softmax-finalize chain
                        if idx < len(iters) - 1:
                            si += 1
                            take = (len(units) * si) // slots_total - ui
                            while take > 0 and ui < len(units):
                                emit_oproj_unit(*units[ui])
                                ui += 1
                                take -= 1
                if po_w is None:
                    po_w = [pq.tile([HD + 1, 512], F32, tag="pq",
                                    name=f"po{ic}_{w}_{hh}")
                            for hh in range(2)]
                while npv < nJ:
                    emit_pv(ic, npv, w, po_w)
                    npv += 1
                fin1(ic, w, po_w)
                if nxt is not None and not inloop_la:
                    emit_scores(nxt[0], 0, nxt[1])
                    emit_scores(nxt[0], 1, nxt[1])
                pending_fin2 = (ic, w, po_w)
            while ui < len(units):
                emit_oproj_unit(*units[ui])
                ui += 1
        # tail: finalize the last wave, then its o_proj via scalar casts
        fin2(*pending_fin2)
        for sb in range(12, 16):
            for n4 in range(NC4):
                emit_oproj_unit(sb, n4, tail=True)

    nc.compile()
    return nc


_NC_CACHE = {}


def _get_module(mm_dt=MM_DT):
    if mm_dt not in _NC_CACHE:
        _NC_CACHE[mm_dt] = _build_module(mm_dt)
    return _NC_CACHE[mm_dt]


def _prep_inputs(x, wq, wk, wv, wo, cos, sin, mm_dt=MM_DT):
    mm_np = mybir.dt.np(mm_dt)
    x = np.asarray(x, dtype=np.float32)
    xT = np.ascontiguousarray(x.reshape(S, H).T.astype(mm_np))

    cosT = np.asarray(cos, dtype=np.float32).T          # [64, S]
    sinT = np.asarray(sin, dtype=np.float32).T          # [64, S]
    sgn = np.where(np.arange(HD) < HD // 2, -1.0, 1.0).astype(np.float32)
    sinT_s = sinT * sgn[:, None]
    cos2 = np.ascontiguousarray(np.tile(cosT, (2, 1))).astype(mm_np)  # [128, S]
    sin2 = np.ascontiguousarray(np.tile(sinT_s, (2, 1))).astype(mm_np)

    # inverted causal masks (1 where masked out), diagonal offsets 0..3
    jl = np.arange(P)[:, None]
    il = np.arange(512)[None, :]
    cminv = np.concatenate(
        [(jl + P * t > il).astype(np.float32) for t in range(4)], axis=1)
    cminv = np.ascontiguousarray(cminv).astype(mm_np)
    negi = (-MASK_NEG * np.eye(P, dtype=np.float32)).astype(mm_np)

    # selector matrix: psb rows 0:64 get the reciprocal row of the wave's
    # first head (partition 0), rows 64:128 the second head (partition 32)
    ew = np.zeros((P, P), dtype=np.float32)
    ew[0, 0:64] = 1.0
    ew[32, 64:128] = 1.0
    ew = ew.astype(mm_np)

    def chunk_kxm(w):
        # [H, M] -> [128, KCH*M] with k-chunk-major free layout
        m = w.shape[1]
        return np.ascontiguousarray(
            w.reshape(KCH, P, m).transpose(1, 0, 2).reshape(P, KCH * m).astype(mm_np))

    wq = np.asarray(wq, dtype=np.float32)
    wk = np.asarray(wk, dtype=np.float32)
    wv = np.asarray(wv, dtype=np.float32)
    wo = np.asarray(wo, dtype=np.float32)

    in_maps = []
    for c in range(NCORES):
        wq_c = wq[:, DQ * c:DQ * c + DQ] * SCALE
        wkv_c = np.concatenate(
            [wk[:, HD * c:HD * c + HD], wv[:, HD * c:HD * c + HD]], axis=1)
        wo_c = wo[DQ * c:DQ * c + DQ, :]
        wo_l = np.ascontiguousarray(
            wo_c.reshape(2, P, H).transpose(1, 0, 2).reshape(P, 2 * H).astype(mm_np))
        in_maps.append({
            "xT": xT,
            "wq": chunk_kxm(wq_c),
            "wkv": chunk_kxm(wkv_c),
            "wo": wo_l,
            "cos2": cos2,
            "sin2": sin2,
            "cminv": cminv,
            "negi": negi,
            "ew": ew,
        })
    return in_maps


def run(inputs, trace=False, trace_kwargs=None, mm_dt=MM_DT):
    """Execute on 8 cores; returns (full_output, BassKernelResults)."""
    nc = _get_module(mm_dt)
    in_maps = _prep_inputs(
        inputs["x"], inputs["wq"], inputs["wk"], inputs["wv"],
        inputs["wo"], inputs["cos"], inputs["sin"], mm_dt=mm_dt)
    kwargs = {}
    if trace:
        kwargs = dict(trace=True, **(trace_kwargs or {}))
    res = run_bass_kernel_spmd(nc, in_maps, core_ids=list(range(NCORES)), **kwargs)
    acc = np.zeros((S, H), dtype=np.float32)
    for c in range(NCORES):
        acc += res.results[c]["out"].astype(np.float32)
    out = acc.reshape(1, S, H)
    return out, res


def kernel(**inputs):
    out, _ = run(inputs, trace=False)
    return out


# revision 23
# speedup vs baseline: 1.2616x; 1.0347x over previous
# Tensor-parallel GQA attention kernel for 8 Trainium2 NeuronCores.
#
# Sharding: each core owns 4 query heads + 1 kv head (32 q / 8 kv heads
# total), computes q/k/v projections for its heads, RoPE, causal
# attention, and a partial o_proj (row slice of wo); the host sums the 8
# partial outputs.
#
# Per-core layout: everything is kept "transposed" ([dim, seq]) so the
# contraction dim of every matmul is the partition axis:
#   qT = wq_c.T @ x.T        [256, S]   (scale folded into wq_c)
#   kvT = wkv_c.T @ x.T      [128, S]   (k rows 0:64, v rows 64:128)
#   scoresT[j, i] = kT.T q   [128-block j, 512-chunk i]  (K=64)
#   causal mask: an extra matmul accumulates -80 * (1 - mask) into the
#   scores psum ((-80 I).T @ cminv), so exp() of masked entries ~ 1e-33
#   attn_T[d, i] = [v|1].T @ exp(scores)   (row 64 = softmax denoms)
#   out_partial[s, :] = attn_T.T-contracted with wo_c rows (fp16 out,
#   host accumulates)
#
# v2 structure: attention runs in two head-WAVES per 512-query chunk
# (wave w covers heads 2w, 2w+1).  A wave's two heads share one
# [128, 1024] 2-bank score-psum tile, so ONE wide (strided) exp per
# (J-block, wave) halves the Scalar-engine instruction count - the
# scalar exp stream is the attention-phase bottleneck.  PSUM is split
# into pool `pq` (4 x 2KB rotating: psq / po accumulators / o_proj /
# selector) and pool `sc` (2 x 4KB: kv-projection pair / score pairs).
# PV matmuls lag scores by 4 J-blocks; the softmax finalize is split in
# two (fin1: DVE reciprocal chain; fin2: selector matmul + normalize)
# emitted a few J-blocks apart so the PE never waits on the reciprocal.
# o_proj units of chunk ic are spread through chunk ic+1's J-loops as
# dense PE filler; the last chunk's units evacuate via the (then-idle)
# Scalar engine.

import sys
from contextlib import ExitStack

for _p in ("/opt/trn_rl_repo", "/root/.axon_site"):
    if _p not in sys.path:
        sys.path.insert(0, _p)

import numpy as np

import concourse.bacc as bacc
import concourse.mybir as mybir
import concourse.tile as tile
from concourse.bass_utils import run_bass_kernel_spmd

F32 = mybir.dt.float32
EXP = mybir.ActivationFunctionType.Exp

# matmul operand dtype: float32r (2 cyc/row, ~1.5e-4/matmul) or
# float16 (1 cyc/row, ~7e-4/matmul)
MM_DT = mybir.dt.float16
MASK_NEG = 80.0

S = 2048          # sequence length
H = 2048          # hidden size
NH = 32           # query heads
NKV = 8           # kv heads
HD = 64           # head dim
NCORES = 8
HPC = NH // NCORES        # query heads per core = 4
DQ = HPC * HD             # per-core q width = 256
SCALE = HD ** -0.5
P = 128
NB = S // P               # 16 128-blocks along seq
NC4 = S // 512            # 4 512-chunks along seq
KCH = H // P              # 16 contraction chunks
HS = S // 2               # half seq


def _build_module(mm_dt):
    nc = bacc.Bacc(trn_type="TRN2", debug=False)

    xT_d = nc.dram_tensor("xT", [H, S], mm_dt, kind="ExternalInput").ap()
    wq_d = nc.dram_tensor("wq", [P, KCH * DQ], mm_dt, kind="ExternalInput").ap()
    wkv_d = nc.dram_tensor("wkv", [P, KCH * P], mm_dt, kind="ExternalInput").ap()
    wo_d = nc.dram_tensor("wo", [P, 2 * S], mm_dt, kind="ExternalInput").ap()
    cos_d = nc.dram_tensor("cos2", [P, S], mm_dt, kind="ExternalInput").ap()
    sin_d = nc.dram_tensor("sin2", [P, S], mm_dt, kind="ExternalInput").ap()
    # inverted causal masks for the 4 diagonal offsets, and -80*I
    cm_d = nc.dram_tensor("cminv", [P, 4 * 512], mm_dt, kind="ExternalInput").ap()
    ni_d = nc.dram_tensor("negi", [P, P], mm_dt, kind="ExternalInput").ap()
    # selector matrix for the denominator broadcast (per wave)
    ew_d = nc.dram_tensor("ew", [P, P], mm_dt, kind="ExternalInput").ap()
    out_d = nc.dram_tensor("out", [S, H], mm_dt, kind="ExternalOutput").ap()

    with tile.TileContext(nc) as tc, ExitStack() as ctx:
        pers = ctx.enter_context(tc.tile_pool(name="pers", bufs=1))

        wq_sb = pers.tile([P, KCH * DQ], mm_dt, tag="wq_sb", name="wq_sb")
        wkv_sb = pers.tile([P, KCH * P], mm_dt, tag="wkv_sb", name="wkv_sb")
        cos_sb = pers.tile([P, S], mm_dt, tag="cos_sb", name="cos_sb")
        sin_sb = pers.tile([P, S], mm_dt, tag="sin_sb", name="sin_sb")
        wo_sb = pers.tile([P, 2 * S], mm_dt, tag="wo_sb", name="wo_sb")
        cm_sb = pers.tile([P, 4 * 512], mm_dt, tag="cm_sb", name="cm_sb")
        ni_sb = pers.tile([P, P], mm_dt, tag="ni_sb", name="ni_sb")
        ew_sb = pers.tile([P, P], mm_dt, tag="ew_sb", name="ew_sb")

        ones16 = pers.tile([P, NB], mm_dt, tag="ones16", name="ones16")
        nc.vector.memset(ones16[:], 1.0)

        # per-half persistent tensors (w == m: wave w covers heads 2w,2w+1)
        qrh = [[pers.tile([P, HS], mm_dt, tag=f"qr{m}_{g}", name=f"qr{m}_{g}")
                for g in range(2)] for m in range(2)]
        ktdh = [pers.tile([P, HS], mm_dt, tag=f"ktd{g}", name=f"ktd{g}")
                for g in range(2)]
        vtmph = [pers.tile([64, HS], mm_dt, tag=f"vtmp{g}", name=f"vtmp{g}")
                 for g in range(2)]
        vbh = [pers.tile([P, 8 * (HD + 1)], mm_dt, tag=f"vb{g}", name=f"vb{g}")
               for g in range(2)]
        attn0 = pers.tile([P, S], mm_dt, tag="attn0", name="attn0")
        attn1 = pers.tile([P, S], mm_dt, tag="attn1", name="attn1")
        attn = [attn0, attn1]
        rsum = pers.tile([P, 512], F32, tag="rsum", name="rsum")
        nc.vector.memset(rsum[:], 1.0)
        rrf = pers.tile([P, 512], F32, tag="rrf", name="rrf")
        rrs = pers.tile([P, 512], F32, tag="rrs", name="rrs")
        rr = pers.tile([P, 512], mm_dt, tag="rr", name="rr")

        for g in range(2):
            vbv = vbh[g][:].rearrange("p (b c) -> p b c", c=HD + 1)
            nc.vector.tensor_copy(vbv[:, :, HD:HD + 1], ones16[:, 0:8])

        # pools; PSUM: pq = 4 x 2KB banks, sc = 2 x 4KB (2-bank) slots
        xp = ctx.enter_context(tc.tile_pool(name="xp", bufs=18))
        rs = ctx.enter_context(tc.tile_pool(name="rs", bufs=2))
        pq = ctx.enter_context(tc.tile_pool(name="pq", bufs=4, space="PSUM"))
        scp = ctx.enter_context(tc.tile_pool(name="scp", bufs=2, space="PSUM"))
        pxp = ctx.enter_context(tc.tile_pool(name="pxp", bufs=9))
        otp = ctx.enter_context(tc.tile_pool(name="otp", bufs=3))

        psq_store = {}
        pskv_store = {}

        # ---------------- projections ----------------
        def proj_mm(g):
            # two passes over the k-chunks: q-matmuls and kv-matmuls are
            # split so the psum handoff to RoPE/the next phase overlaps
            # with the other projection's matmuls (g=0: q first, so the
            # psq banks release during the kv pass; g=1: kv first, so the
            # g=0 rope casts finish during the kv pass)
            nw = slice(1024 * g, 1024 * g + 1024)
            psq = [[pq.tile([P, 512], F32, tag="pq", name=f"psq{m}_{half}")
                    for half in range(2)] for m in range(2)]
            pskv = scp.tile([P, 1024], F32, tag="sc", name="pskv")
            psq_store[g] = psq
            pskv_store[g] = pskv

            def emit_q(k, xt):
                st = dict(start=(k == 0), stop=(k == KCH - 1))
                for half in range(2):
                    xs = xt[:, 512 * half:512 * half + 512]
                    nc.tensor.matmul(
                        psq[0][half][:], wq_sb[:, k * DQ:k * DQ + P], xs, **st)
                    nc.tensor.matmul(
                        psq[1][half][:], wq_sb[:, k * DQ + P:k * DQ + DQ],
                        xs, **st)

            def emit_kv(k, xt):
                st = dict(start=(k == 0), stop=(k == KCH - 1))
                for half in range(2):
                    xs = xt[:, 512 * half:512 * half + 512]
                    nc.tensor.matmul(
                        pskv[:, 512 * half:512 * half + 512],
                        wkv_sb[:, k * P:k * P + P], xs, **st)

            xts = []
            for k in range(KCH):
                if g == 0 and (k in (0, 1) or (k % 4 == 2 and k < 12)):
                    # k=0/1 come alone so the first matmuls start early
                    if k == 0:
                        wqs, wks = slice(0, 256), slice(0, 128)
                    elif k == 1:
                        wqs, wks = slice(256, 1024), slice(128, 512)
                    else:
                        kg = k // 4 + 1
                        wqs = slice(1024 * kg, 1024 * kg + 1024)
                        wks = slice(512 * kg, 512 * kg + 512)
                    nc.sync.dma_start(wq_sb[:, wqs], wq_d[:, wqs])
                    nc.sync.dma_start(wkv_sb[:, wks], wkv_d[:, wks])
                xt = xp.tile([P, 1024], mm_dt, tag="xt", name="xt")
                if g == 0 and k < 2:
                    # split the first x tiles so the first matmul's data
                    # lands sooner
                    nc.sync.dma_start(
                        xt[:, 0:512], xT_d[P * k:P * k + P, 0:512])
                    nc.sync.dma_start(
                        xt[:, 512:1024], xT_d[P * k:P * k + P, 512:1024])
                else:
                    nc.sync.dma_start(xt[:], xT_d[P * k:P * k + P, nw])
                xts.append(xt)
                if g == 0:
                    emit_q(k, xt)
                else:
                    emit_kv(k, xt)
                if g == 0 and k == 3:
                    # trig tables needed by the first RoPE below
                    nc.sync.dma_start(cos_sb[:], cos_d[:])
                    nc.sync.dma_start(sin_sb[:], sin_d[:])
                if g == 0 and k == 9:
                    nc.sync.dma_start(cm_sb[:], cm_d[:])
                    nc.sync.dma_start(ni_sb[:], ni_d[:])
                    nc.sync.dma_start(ew_sb[:], ew_d[:])
                if g == 1 and k == 4:
                    # wo is first needed by chunk-1's o_proj fillers; keep
                    # its 2MB out of the projection-critical DMA window
                    nc.sync.dma_start(wo_sb[:], wo_d[:])
            for k in range(KCH):
                if g == 0:
                    emit_kv(k, xts[k])
                else:
                    emit_q(k, xts[k])

        # ---------------- RoPE ----------------
        def rope(g):
            # PSUM evacuation is split across Scalar and DVE (fp16
            # casts); for g=0 the q casts go first (the psq slots gate
            # the g=1 projection), for g=1 the kv casts go on DVE (they
            # gate the chunk-0 score tiles and must not queue behind the
            # chunk-0 exps on the scalar engine).  Then an SBUF-only
            # fp16 rotate-half chain on DVE.
            nw = slice(1024 * g, 1024 * g + 1024)
            psq = psq_store.pop(g)
            pskv = pskv_store.pop(g)
            q16s = []
            for m in range(2):
                q16 = rs.tile([P, 1024], mm_dt, tag=f"q16_{m}", name=f"q16_{m}")
                q16s.append(q16)
                eng = nc.vector if (g == 0 and m == 0) else nc.scalar
                for half in range(2):
                    hs = slice(512 * half, 512 * half + 512)
                    if eng is nc.vector:
                        nc.vector.tensor_copy(q16[:, hs], psq[m][half][:])
                    else:
                        nc.scalar.copy(q16[:, hs], psq[m][half][:])
            k16 = rs.tile([64, 1024], mm_dt, tag="k16", name="k16")
            for half in range(2):
                hs = slice(512 * half, 512 * half + 512)
                if g == 1:
                    nc.vector.tensor_copy(k16[:, hs], pskv[0:64, hs])
                else:
                    nc.scalar.copy(k16[:, hs], pskv[0:64, hs])
            for half in range(2):
                hs = slice(512 * half, 512 * half + 512)
                if g == 1:
                    nc.vector.tensor_copy(vtmph[g][:, hs], pskv[64:P, hs])
                else:
                    nc.scalar.copy(vtmph[g][:, hs], pskv[64:P, hs])
            # fp16 SBUF chains on DVE
            for m in range(2):
                q16 = q16s[m]
                nc.vector.tensor_mul(qrh[m][g][:], q16[:], cos_sb[:, nw])
                qsw = rs.tile([P, 1024], mm_dt, tag=f"qsw{m}", name=f"qsw{m}")
                for b0 in (0, 64):
                    nc.vector.tensor_copy(
                        qsw[b0:b0 + 32, :], q16[b0 + 32:b0 + 64, :])
                    nc.vector.tensor_copy(
                        qsw[b0 + 32:b0 + 64, :], q16[b0:b0 + 32, :])
                nc.vector.tensor_mul(qsw[:], qsw[:], sin_sb[:, nw])
                nc.vector.tensor_add(qrh[m][g][:], qrh[m][g][:], qsw[:])
            nc.vector.tensor_mul(ktdh[g][0:64, :], k16[:], cos_sb[0:64, nw])
            ksw = rs.tile([64, 1024], mm_dt, tag="ksw", name="ksw")
            nc.vector.tensor_copy(ksw[0:32, :], k16[32:64, :])
            nc.vector.tensor_copy(ksw[32:64, :], k16[0:32, :])
            nc.vector.tensor_mul(ksw[:], ksw[:], sin_sb[0:64, nw])
            nc.vector.tensor_add(ktdh[g][0:64, :], ktdh[g][0:64, :], ksw[:])
            # duplicate k rows for the upper-head score matmuls
            nc.vector.tensor_copy(ktdh[g][64:P, :], ktdh[g][0:64, :])
            # stream-transpose v into vb blocks (DVE)
            vbv = vbh[g][:].rearrange("p (b c) -> p b c", c=HD + 1)
            vtv = vtmph[g][:].rearrange("p (b c) -> p b c", c=P)
            for a in range(4):
                for b in range(2):
                    nc.vector.transpose(
                        vbv[32 * a:32 * a + 32, :, 32 * b:32 * b + 32],
                        vtv[32 * b:32 * b + 32, :, 32 * a:32 * a + 32])

        # ---------------- attention ----------------
        pxstore = {}

        def emit_scores(ic, J, w):
            icg, icr = ic // 2, ic % 2
            t = J - 4 * ic
            c0 = 128 * t if t > 0 else 0
            Jg, Jr = J // 8, J % 8
            Js = slice(P * Jr, P * Jr + P)
            qs = slice(512 * icr + c0, 512 * icr + 512)
            ps_s = scp.tile([P, 1024], F32, tag="sc", name="ps_s")
            for hh in range(2):
                b0, col = 64 * hh, 512 * hh
                nc.tensor.matmul(
                    ps_s[:, col + c0:col + 512], ktdh[Jg][b0:b0 + 64, Js],
                    qrh[w][icg][b0:b0 + 64, qs],
                    start=True, stop=(t < 0))
            if t >= 0:
                for hh in range(2):
                    col = 512 * hh
                    nc.tensor.matmul(
                        ps_s[:, col + c0:col + 512], ni_sb[:],
                        cm_sb[:, 512 * t + c0:512 * t + 512],
                        start=False, stop=True)
            px = pxp.tile([P, 1024], mm_dt, tag="pxp", name="px")
            # one strided exp covering both heads' live regions
            ps_v = ps_s[:].rearrange("p (two c) -> p two c", two=2)
            px_v = px[:].rearrange("p (two c) -> p two c", two=2)
            nc.scalar.activation(px_v[:, :, c0:], ps_v[:, :, c0:], EXP)
            pxstore[(ic, J, w)] = px

        def emit_pv(ic, J, w, po_w):
            t = J - 4 * ic
            c0 = 128 * t if t > 0 else 0
            nJ = 4 * ic + 4
            Jg, Jr = J // 8, J % 8
            vs = slice((HD + 1) * Jr, (HD + 1) * Jr + HD + 1)
            px = pxstore.pop((ic, J, w))
            for hh in range(2):
                col = 512 * hh
                nc.tensor.matmul(
                    po_w[hh][:, c0:], vbh[Jg][:, vs], px[:, col + c0:col + 512],
                    start=(J == 0), stop=(J == nJ - 1))

        def fin1(ic, w, po_w):
            # DVE-only: gather the 2 denominator rows, batched reciprocal
            # (fast variant: ~18 correct bits, far above the fp16 rr cast)
            for hh in range(2):
                nc.vector.tensor_copy(
                    rsum[32 * hh:32 * hh + 1, :], po_w[hh][HD:HD + 1, :])
            with nc.allow_low_precision(reason="softmax reciprocal"):
                nc.vector.reciprocal_approx_fast(rrf[:], rsum[:])
                nc.vector.tensor_copy(rr[:], rrf[:])

        def fin2(ic, w, po_w):
            # selector matmul broadcasts the reciprocals, then normalize
            psb = pq.tile([P, 512], F32, tag="pq", name="psb")
            nc.tensor.matmul(psb[:], ew_sb[:], rr[:], start=True, stop=True)
            psbs = rs.tile([P, 512], F32, tag="psbs", name="psbs")
            nc.vector.tensor_copy(psbs[:], psb[:])
            for hh in range(2):
                b0 = 64 * hh
                asl = attn[w][b0:b0 + 64, 512 * ic:512 * ic + 512]
                nc.vector.tensor_mul(asl, po_w[hh][0:HD, :], psbs[b0:b0 + 64, :])

        ot_store = {}

        def emit_oproj_unit(sb, n4, tail=False):
            ss = slice(P * sb, P * sb + P)
            ps_o = pq.tile([P, 512], F32, tag="pq", name="ps_o")
            nc.tensor.matmul(
                ps_o[:], attn0[:, ss],
                wo_sb[:, 512 * n4:512 * n4 + 512],
                start=True, stop=False)
            nc.tensor.matmul(
                ps_o[:], attn1[:, ss],
                wo_sb[:, S + 512 * n4:S + 512 * n4 + 512],
                start=False, stop=True)
            if n4 == 0:
                ot_store[sb] = otp.tile([P, H], mm_dt, tag="otp", name="ot")
            ot = ot_store[sb]
            osl = slice(512 * n4, 512 * n4 + 512)
            if tail:
                # both scalar and DVE are idle at the tail: alternate casts
                if n4 % 2 == 0:
                    nc.scalar.copy(ot[:, osl], ps_o[:])
                else:
                    nc.vector.tensor_copy(ot[:, osl], ps_o[:])
            else:
                nc.vector.tensor_copy(ot[:, osl], ps_o[:])
            if n4 == NC4 - 1:
                nc.sync.dma_start(out_d[ss, :], ot_store.pop(sb)[:])

        # ---------------- emission ----------------
        proj_mm(0)
        rope(0)
        proj_mm(1)
        # chunk-0's first score groups (and their exps) are emitted before
        # rope(1) so the scalar engine starts the exp stream immediately
        # after the g=1 projection instead of behind rope(1)'s casts
        emit_scores(0, 0, 0)
        emit_scores(0, 1, 0)
        rope(1)
        pending_fin2 = None
        units = []
        ui = si = 0
        slots_total = 1

        for ic in range(NC4):
            nJ = 4 * ic + 4
            # filler units: o_proj of the previous chunk, spread over this
            # chunk's score/PV loop iterations (skipping the first two of
            # each wave, which cover fin2 / chunk-boundary latency)
            units = [(sb, n4) for sb in range(4 * (ic - 1), 4 * ic)
                     for n4 in range(NC4)] if ic > 0 else []
            ui = si = 0
            slots_total = max(1, 2 * (nJ - 5))
            for w in range(2):
                po_w = None
                npv = 0
                iters = list(range(2, nJ))
                # lookahead target: first two score groups of the next
                # wave/chunk, emitted mid-loop (long waves) so their exps
                # are already drained when the next wave's PVs need them
                if w == 0:
                    nxt = (ic, 1)
                elif ic + 1 < NC4:
                    nxt = (ic + 1, 0)
                else:
                    nxt = None
                inloop_la = len(iters) >= 6
                for idx, J in enumerate(iters):
                    emit_scores(ic, J, w)
                    if idx == 1 and pending_fin2 is not None:
                        fin2(*pending_fin2)
                        pending_fin2 = None
                    if idx >= 2:
                        if po_w is None:
                            po_w = [pq.tile([HD + 1, 512], F32, tag="pq",
                                            name=f"po{ic}_{w}_{hh}")
                                    for hh in range(2)]
                        emit_pv(ic, npv, w, po_w)
                        npv += 1
                        if inloop_la and nxt is not None and \
                                idx in (len(iters) - 4, len(iters) - 3):
                            emit_scores(nxt[0], idx - (len(iters) - 4), nxt[1])
                        # fillers skip the wave's last iteration so the
                        # DVE is free for the softmax-finalize chain
                        if idx < len(iters) - 1:
                            si += 1
                            take = (len(units) * si) // slots_total - ui
                            while take > 0 and ui < len(units):
                                emit_oproj_unit(*units[ui])
                                ui += 1
                                take -= 1
                if po_w is None:
                    po_w = [pq.tile([HD + 1, 512], F32, tag="pq",
                                    name=f"po{ic}_{w}_{hh}")
                            for hh in range(2)]
                while npv < nJ:
                    emit_pv(ic, npv, w, po_w)
                    npv += 1
                fin1(ic, w, po_w)
                if nxt is not None and not inloop_la:
                    emit_scores(nxt[0], 0, nxt[1])
                    emit_scores(nxt[0], 1, nxt[1])
                pending_fin2 = (ic, w, po_w)
            while ui < len(units):
                emit_oproj_unit(*units[ui])
                ui += 1
        # tail: finalize the last wave, then its o_proj via scalar casts
        fin2(*pending_fin2)
        for sb in range(12, 16):
            for n4 in range(NC4):
                emit_oproj_unit(sb, n4, tail=True)

    nc.compile()
    return nc


_NC_CACHE = {}


def _get_module(mm_dt=MM_DT):
    if mm_dt not in _NC_CACHE:
        _NC_CACHE[mm_dt] = _build_module(mm_dt)
    return _NC_CACHE[mm_dt]


def _prep_inputs(x, wq, wk, wv, wo, cos, sin, mm_dt=MM_DT):
    mm_np = mybir.dt.np(mm_dt)
    x = np.asarray(x, dtype=np.float32)
    xT = np.ascontiguousarray(x.reshape(S, H).T.astype(mm_np))

    cosT = np.asarray(cos, dtype=np.float32).T          # [64, S]
    sinT = np.asarray(sin, dtype=np.float32).T          # [64, S]
    sgn = np.where(np.arange(HD) < HD // 2, -1.0, 1.0).astype(np.float32)
    sinT_s = sinT * sgn[:, None]
    cos2 = np.ascontiguousarray(np.tile(cosT, (2, 1))).astype(mm_np)  # [128, S]
    sin2 = np.ascontiguousarray(np.tile(sinT_s, (2, 1))).astype(mm_np)

    # inverted causal masks (1 where masked out), diagonal offsets 0..3
    jl = np.arange(P)[:, None]
    il = np.arange(512)[None, :]
    cminv = np.concatenate(
        [(jl + P * t > il).astype(np.float32) for t in range(4)], axis=1)
    cminv = np.ascontiguousarray(cminv).astype(mm_np)
    negi = (-MASK_NEG * np.eye(P, dtype=np.float32)).astype(mm_np)

    # selector matrix: psb rows 0:64 get the reciprocal row of the wave's
    # first head (partition 0), rows 64:128 the second head (partition 32)
    ew = np.zeros((P, P), dtype=np.float32)
    ew[0, 0:64] = 1.0
    ew[32, 64:128] = 1.0
    ew = ew.astype(mm_np)

    def chunk_kxm(w):
        # [H, M] -> [128, KCH*M] with k-chunk-major free layout
        m = w.shape[1]
        return np.ascontiguousarray(
            w.reshape(KCH, P, m).transpose(1, 0, 2).reshape(P, KCH * m).astype(mm_np))

    wq = np.asarray(wq, dtype=np.float32)
    wk = np.asarray(wk, dtype=np.float32)
    wv = np.asarray(wv, dtype=np.float32)
    wo = np.asarray(wo, dtype=np.float32)

    in_maps = []
    for c in range(NCORES):
        wq_c = wq[:, DQ * c:DQ * c + DQ] * SCALE
        wkv_c = np.concatenate(
            [wk[:, HD * c:HD * c + HD], wv[:, HD * c:HD * c + HD]], axis=1)
        wo_c = wo[DQ * c:DQ * c + DQ, :]
        wo_l = np.ascontiguousarray(
            wo_c.reshape(2, P, H).transpose(1, 0, 2).reshape(P, 2 * H).astype(mm_np))
        in_maps.append({
            "xT": xT,
            "wq": chunk_kxm(wq_c),
            "wkv": chunk_kxm(wkv_c),
            "wo": wo_l,
            "cos2": cos2,
            "sin2": sin2,
            "cminv": cminv,
            "negi": negi,
            "ew": ew,
        })
    return in_maps


def run(inputs, trace=False, trace_kwargs=None, mm_dt=MM_DT):
    """Execute on 8 cores; returns (full_output, BassKernelResults)."""
    nc = _get_module(mm_dt)
    in_maps = _prep_inputs(
        inputs["x"], inputs["wq"], inputs["wk"], inputs["wv"],
        inputs["wo"], inputs["cos"], inputs["sin"], mm_dt=mm_dt)
    kwargs = {}
    if trace:
        kwargs = dict(trace=True, **(trace_kwargs or {}))
    res = run_bass_kernel_spmd(nc, in_maps, core_ids=list(range(NCORES)), **kwargs)
    acc = np.zeros((S, H), dtype=np.float32)
    for c in range(NCORES):
        acc += res.results[c]["out"].astype(np.float32)
    out = acc.reshape(1, S, H)
    return out, res


def kernel(**inputs):
    out, _ = run(inputs, trace=False)
    return out
